# revision 5
# baseline (speedup 1.0000x reference)
"""
2-layer GAT on Trainium2 (8 NeuronCores, SPMD via bass/Tile).

Sharding: destination nodes are block-sharded across the 8 cores (6250
nodes each).  All per-edge work runs on the core owning the edge's dst.

Pipeline = three bass kernels with host gathers between them:
  A1: node phase, sharded — core c computes h = x@W1pack for its own
      ~6272 nodes only, writes t1slab [h|d1] (bf16) + sslab s1 (bf16).
      Host assembles the full table1 from the 8 slabs.
  A2: layer-0 edge phase.  Per-edge h|d1 rows come from one
      dma_gather per edge (512 B rows, lo/hi split for int16 idx).
      s1[dst] is NOT gathered: dst rows of a tile are contiguous, so a
      [128,8] s-tile is DMA'd and broadcast per edge on the PE via
      S_T = transpose(S) followed by se = S_T.T @ s_tile (exact: S is
      one-hot).  Aggregation per chunk of 128 edges: S.T @ [feat*ex|ex]
      accumulated in PSUM; then normalize, ELU, W2 matmul -> table2slab.
  B:  layer-1 edge phase.  feat2|d2[src] via one pair-row dma_gather
      per edge (int16 idx = src>>1 + parity select).  s2[dst] via the
      same PE one-hot broadcast trick.  log_softmax tail.

Softmax max-subtraction is skipped: logits are O(0.3) so exp() is
stable, and softmax is shift-invariant.
"""

import os
import sys
from contextlib import ExitStack

import numpy as np
import ml_dtypes

for _p in ("/opt/trn_rl_repo",):
    if os.path.isdir(_p) and _p not in sys.path:
        sys.path.insert(0, _p)

import concourse.bass as bass
import concourse.bacc as bacc
import concourse.tile as tile
from concourse import mybir
from concourse import bass_utils
from concourse._compat import with_exitstack

F32 = mybir.dt.float32
BF16 = mybir.dt.bfloat16
I32 = mybir.dt.int32
I16 = mybir.dt.int16
AF = mybir.ActivationFunctionType
OP = mybir.AluOpType
P = 128
BF = ml_dtypes.bfloat16


class Cfg:
    def __init__(self, N, E, ncores, split=32768, neg=0.2, in_ch=128,
                 f=128, heads=8, hid=16, out=16):
        self.N = N
        self.E = E
        self.NCORES = ncores
        self.SPLIT = split
        self.NEG = neg
        self.IN = in_ch
        self.F = f
        self.H = heads
        self.HID = hid
        self.OUT = out
        assert N % ncores == 0
        self.NPC = N // ncores
        self.TPC = (self.NPC + P - 1) // P
        self.NPC_PAD = self.TPC * P
        self.NTILES = ncores * self.TPC
        self.N_PAD = self.NTILES * P
        self.NCHL_T = None
        self.NCHH_T = None
        self.NCH_T = None
        self.NCH = None


def _wrap16(vals):
    """[n] slot-ordered values -> [128, n//16] int16 wrapped layout."""
    n = vals.shape[0]
    assert n % 16 == 0
    w = vals.reshape(-1, 16).T.astype(np.int16)      # [16, n//16]
    return np.ascontiguousarray(np.tile(w, (8, 1)))  # [128, n//16]


def _prep_graph(cfg, edge_index):
    N, NPC, SPL = cfg.N, cfg.NPC, cfg.SPLIT
    src = np.concatenate([edge_index[0], np.arange(N, dtype=np.int64)])
    dst = np.concatenate([edge_index[1], np.arange(N, dtype=np.int64)])
    core = dst // NPC
    ld = dst - core * NPC
    tile_id = ld // P
    dstloc = ld % P
    hi = (src >= SPL).astype(np.int64)
    order = np.lexsort((src, hi, tile_id, core))
    src, dst, core, tile_id, dstloc, hi = (a[order] for a in
                                           (src, dst, core, tile_id,
                                            dstloc, hi))
    keyf = (core * cfg.TPC + tile_id) * 2 + hi
    cntf = np.bincount(keyf, minlength=cfg.NCORES * cfg.TPC * 2)
    cnt_lo = cntf[0::2].reshape(cfg.NCORES, cfg.TPC)
    cnt_hi = cntf[1::2].reshape(cfg.NCORES, cfg.TPC)
    cfg.NCHL_T = max(1, int(np.max((cnt_lo + P - 1) // P)))
    cfg.NCHH_T = max(1, int(np.max((cnt_hi + P - 1) // P)))
    cfg.NCH_T = cfg.NCHL_T + cfg.NCHH_T
    cfg.NCH = cfg.NCH_T * cfg.TPC
    starts = np.concatenate([[0], np.cumsum(cntf)])

    pc = dict(srcw_lo=[], srcw_hi=[], srcw=[], dstloc_f=[], src_par=[])
    for c in range(cfg.NCORES):
        ns_lo = cfg.TPC * cfg.NCHL_T * P
        ns_hi = cfg.TPC * cfg.NCHH_T * P
        ns = cfg.NCH * P
        v_srclo = np.zeros(ns_lo, np.int64)
        v_srchi = np.zeros(ns_hi, np.int64)
        v_src = np.zeros(ns, np.int64)          # src>>1 per slot
        v_dstloc = np.full(ns, -1.0, np.float32)
        v_spar = np.ones(ns, np.float32)
        for t in range(cfg.TPC):
            for h in (0, 1):
                k = ((c * cfg.TPC + t) * 2 + h)
                n = int(cntf[k])
                if n == 0:
                    continue
                sl = slice(starts[k], starts[k] + n)
                e_src = src[sl]
                e_dl = dstloc[sl]
                pos = np.arange(n)
                if h == 0:
                    v_srclo[t * cfg.NCHL_T * P + pos] = e_src
                    ch = t * cfg.NCH_T + pos // P
                else:
                    v_srchi[t * cfg.NCHH_T * P + pos] = e_src - SPL
                    ch = t * cfg.NCH_T + cfg.NCHL_T + pos // P
                slot = ch * P + pos % P
                v_src[slot] = e_src >> 1
                v_dstloc[slot] = e_dl
                v_spar[slot] = 1.0 - (e_src & 1)
        pc["srcw_lo"].append(_wrap16(v_srclo))
        pc["srcw_hi"].append(_wrap16(v_srchi))
        pc["srcw"].append(_wrap16(v_src))
        # slot arrays in [128, NCH] layout: slot = ch*128 + p -> [p, ch]
        pc["dstloc_f"].append(
            np.ascontiguousarray(v_dstloc.reshape(cfg.NCH, P).T))
        pc["src_par"].append(np.ascontiguousarray(
            v_spar.reshape(cfg.NCH, P).T))
    return pc


def _blockdiag_att(att, heads, hid, f):
    A = np.zeros((f, heads), dtype=np.float32)
    for h in range(heads):
        A[h * hid:(h + 1) * hid, h] = att[0, h]
    return A


def _ap(base, ap_list, off_extra=0):
    return bass.AP(tensor=base.tensor, offset=base.offset + off_extra,
                   ap=ap_list)


@with_exitstack
def _build_a1(ctx, tc, cfg, t):
    """Sharded node phase: this core's TPC tiles only."""
    nc = tc.nc
    WCOLS = cfg.F + 2 * cfg.H             # 144 matmul out cols
    TCOLS = cfg.F + cfg.H                 # 136 table1 used cols

    consts = ctx.enter_context(tc.tile_pool(name="consts", bufs=1))
    wpack = consts.tile([P, WCOLS], F32)
    nc.sync.dma_start(out=wpack[:], in_=t["wpack"][:, :])

    NT = cfg.TPC
    BLK = 8
    xpool = ctx.enter_context(tc.tile_pool(name="xt", bufs=2))
    npsum = ctx.enter_context(tc.tile_pool(name="npsum", bufs=2,
                                           space="PSUM"))
    nstage = ctx.enter_context(tc.tile_pool(name="nstage", bufs=3))
    for blk in range((NT + BLK - 1) // BLK):
        nt0 = blk * BLK
        nt1 = min(nt0 + BLK, NT)
        xt = xpool.tile([P, BLK * P], F32, tag="xt")
        nc.sync.dma_start(out=xt[:, 0:(nt1 - nt0) * P],
                          in_=t["xTs"][:, nt0 * P:nt1 * P])
        for j in range(nt1 - nt0):
            nt = nt0 + j
            pt = npsum.tile([P, WCOLS], F32, tag="npt")
            nc.tensor.matmul(out=pt[:], lhsT=xt[:, j * P:(j + 1) * P],
                             rhs=wpack[:], start=True, stop=True)
            s1 = nstage.tile([P, TCOLS], BF16, tag="s1")
            nc.scalar.activation(s1[:], pt[:, 0:TCOLS], AF.Copy)
            ss = nstage.tile([P, cfg.H], BF16, tag="ss")
            nc.scalar.activation(ss[:], pt[:, TCOLS:WCOLS], AF.Copy)
            nc.sync.dma_start(
                out=t["t1slab"][nt * P:(nt + 1) * P, 0:TCOLS], in_=s1[:])
            nc.sync.dma_start(
                out=t["sslab"][nt * P:(nt + 1) * P, 0:cfg.H], in_=ss[:])


@with_exitstack
def _build_a2(ctx, tc, cfg, t):
    nc = tc.nc
    NCH_T, NCHL_T, NCHH_T, TPC = cfg.NCH_T, cfg.NCHL_T, cfg.NCHH_T, cfg.TPC
    MCOLS = cfg.F + cfg.H                 # 136 message cols
    ROW1 = 256                            # table1 row elems (bf16)
    W2COLS = cfg.OUT + 2

    consts = ctx.enter_context(tc.tile_pool(name="consts", bufs=1))
    w2pack = consts.tile([P, W2COLS], F32)
    nc.sync.dma_start(out=w2pack[:], in_=t["w2pack"][:, :])
    iota = consts.tile([P, P], F32)
    nc.sync.dma_start(out=iota[:], in_=t["iota"][:, :])
    ident = consts.tile([P, P], F32)
    nc.sync.dma_start(out=ident[:], in_=t["ident"][:, :])
    identb = consts.tile([P, P], BF16)
    nc.sync.dma_start(out=identb[:], in_=t["identb"][:, :])

    gpool = ctx.enter_context(tc.tile_pool(name="g", bufs=2))
    ipool = ctx.enter_context(tc.tile_pool(name="idx", bufs=2))
    spool = ctx.enter_context(tc.tile_pool(name="sel", bufs=2))
    mpool = ctx.enter_context(tc.tile_pool(name="msg", bufs=2))
    lpool = ctx.enter_context(tc.tile_pool(name="logit", bufs=2))
    apsum = ctx.enter_context(tc.tile_pool(name="apsum", bufs=2,
                                           space="PSUM"))
    stpsum = ctx.enter_context(tc.tile_pool(name="stpsum", bufs=2,
                                            space="PSUM"))
    sepsum = ctx.enter_context(tc.tile_pool(name="sepsum", bufs=2,
                                            space="PSUM"))
    tpsum = ctx.enter_context(tc.tile_pool(name="tpsum", bufs=1,
                                           space="PSUM"))
    t2psum = ctx.enter_context(tc.tile_pool(name="t2psum", bufs=1,
                                            space="PSUM"))
    stsb = ctx.enter_context(tc.tile_pool(name="stsb", bufs=3))
    hpool = ctx.enter_context(tc.tile_pool(name="h1", bufs=2))

    tab_hi = t["table1"][cfg.SPLIT:cfg.N_PAD, :]

    for ti in range(TPC):
        c0 = ti * NCH_T
        il = ipool.tile([P, NCHL_T * 8], I16, tag="il")
        nc.sync.dma_start(out=il[:], in_=t["srcw_lo"][
            :, ti * NCHL_T * 8:(ti + 1) * NCHL_T * 8])
        ih = ipool.tile([P, NCHH_T * 8], I16, tag="ih")
        nc.sync.dma_start(out=ih[:], in_=t["srcw_hi"][
            :, ti * NCHH_T * 8:(ti + 1) * NCHH_T * 8])
        dloc = ipool.tile([P, NCH_T], F32, tag="dloc")
        nc.sync.dma_start(out=dloc[:], in_=t["dstloc_f"][:, c0:c0 + NCH_T])
        s_tile = ipool.tile([P, cfg.H], BF16, tag="stile")
        nc.sync.dma_start(out=s_tile[:],
                          in_=t["sslab"][ti * P:(ti + 1) * P, 0:cfg.H])

        # gathers (h|d1 by src)
        G = gpool.tile([P, NCH_T, ROW1], BF16, tag="G")
        nc.gpsimd.dma_gather(
            out_ap=G[:, 0:NCHL_T, :], in_ap=t["table1"][:, :],
            idxs_ap=il[:], num_idxs=NCHL_T * P, num_idxs_reg=NCHL_T * P,
            elem_size=ROW1, single_packet=False)
        nc.gpsimd.dma_gather(
            out_ap=G[:, NCHL_T:NCH_T, :], in_ap=tab_hi,
            idxs_ap=ih[:], num_idxs=NCHH_T * P, num_idxs_reg=NCHH_T * P,
            elem_size=ROW1, single_packet=False)

        # one-hot S
        S = spool.tile([P, NCH_T, P], BF16, tag="S")
        nc.vector.tensor_tensor(
            out=S[:],
            in0=_ap(iota[:], [iota[:].ap[0], [0, NCH_T], [1, P]]),
            in1=_ap(dloc[:], [dloc[:].ap[0], [1, NCH_T], [0, P]]),
            op=OP.is_equal)

        # s1[dst] per edge via PE: se = (S_k)^T.T @ s_tile
        sesb = spool.tile([P, NCH_T, cfg.H], F32, tag="sesb")
        for k in range(NCH_T):
            st_ps = stpsum.tile([P, P], BF16, tag="st")
            nc.tensor.transpose(out=st_ps[:], in_=S[:, k, :],
                                identity=identb[:])
            st_sb = stsb.tile([P, P], BF16, tag="stsb")
            nc.scalar.activation(st_sb[:], st_ps[:], AF.Copy)
            se_ps = sepsum.tile([P, cfg.H], F32, tag="se")
            nc.tensor.matmul(out=se_ps[:], lhsT=st_sb[:], rhs=s_tile[:],
                             start=True, stop=True)
            nc.scalar.activation(sesb[:, k, :], se_ps[:], AF.Copy)

        # logits -> ex (bf16)
        dcp = lpool.tile([P, NCH_T, cfg.H], F32, tag="dcp")
        nc.scalar.activation(dcp[:], G[:, :, cfg.F:cfg.F + cfg.H], AF.Copy)
        u = lpool.tile([P, NCH_T, cfg.H], F32, tag="u")
        nc.vector.tensor_tensor(out=u[:], in0=sesb[:], in1=dcp[:], op=OP.add)
        a = lpool.tile([P, NCH_T, cfg.H], F32, tag="a")
        nc.vector.scalar_tensor_tensor(out=a[:], in0=u[:], scalar=cfg.NEG,
                                       in1=u[:], op0=OP.mult, op1=OP.max)
        ex = lpool.tile([P, NCH_T, cfg.H], BF16, tag="ex")
        nc.scalar.activation(ex[:], a[:], AF.Exp)

        # Msg = [feat * ex | ex]
        M = mpool.tile([P, NCH_T, MCOLS], BF16, tag="M")
        nc.scalar.activation(M[:, :, cfg.F:MCOLS], ex[:], AF.Copy)
        nc.vector.tensor_tensor(
            out=_ap(M[:], [M[:].ap[0], [MCOLS, NCH_T], [cfg.HID, cfg.H],
                           [1, cfg.HID]]),
            in0=_ap(G[:], [G[:].ap[0], [ROW1, NCH_T], [cfg.HID, cfg.H],
                           [1, cfg.HID]]),
            in1=_ap(ex[:], [ex[:].ap[0], [cfg.H, NCH_T], [1, cfg.H],
                            [0, cfg.HID]]),
            op=OP.mult)

        # aggregate
        agg = apsum.tile([P, MCOLS], F32, tag="agg")
        for k in range(NCH_T):
            nc.tensor.matmul(out=agg[:], lhsT=S[:, k, :], rhs=M[:, k, :],
                             start=(k == 0), stop=(k == NCH_T - 1))

        # normalize + elu + feat2/d2/s2 slab
        den = hpool.tile([P, cfg.H], F32, tag="den")
        nc.vector.tensor_scalar_add(den[:], agg[:, cfg.F:MCOLS], 1e-20)
        rcp = hpool.tile([P, cfg.H], F32, tag="rcp")
        nc.vector.reciprocal(rcp[:], den[:])
        h1 = hpool.tile([P, cfg.F], F32, tag="h1")
        nc.vector.tensor_tensor(
            out=_ap(h1[:], [h1[:].ap[0], [cfg.HID, cfg.H], [1, cfg.HID]]),
            in0=_ap(agg[:], [agg[:].ap[0], [cfg.HID, cfg.H], [1, cfg.HID]]),
            in1=_ap(rcp[:], [rcp[:].ap[0], [1, cfg.H], [0, cfg.HID]]),
            op=OP.mult)
        neg = hpool.tile([P, cfg.F], F32, tag="neg")
        nc.vector.tensor_scalar_min(neg[:], h1[:], 0.0)
        pos = hpool.tile([P, cfg.F], F32, tag="pos")
        nc.vector.tensor_scalar_max(pos[:], h1[:], 0.0)
        een = hpool.tile([P, cfg.F], F32, tag="een")
        nc.scalar.activation(een[:], neg[:], AF.Exp)
        elu = hpool.tile([P, cfg.F], F32, tag="elu")
        nc.vector.scalar_tensor_tensor(out=elu[:], in0=een[:], scalar=-1.0,
                                       in1=pos[:], op0=OP.add, op1=OP.add)
        eT_ps = tpsum.tile([P, P], F32, tag="eT")
        nc.tensor.transpose(out=eT_ps[:], in_=elu[:], identity=ident[:])
        eT = hpool.tile([P, P], F32, tag="eTs")
        nc.scalar.activation(eT[:], eT_ps[:], AF.Copy)
        t2 = t2psum.tile([P, W2COLS], F32, tag="t2")
        nc.tensor.matmul(out=t2[:], lhsT=eT[:], rhs=w2pack[:],
                         start=True, stop=True)
        t2s = hpool.tile([P, W2COLS], F32, tag="t2s")
        nc.scalar.activation(t2s[:], t2[:], AF.Copy)
        nc.sync.dma_start(out=t["table2slab"][ti * P:(ti + 1) * P, :],
                          in_=t2s[:])


@with_exitstack
def _build_b(ctx, tc, cfg, t):
    nc = tc.nc
    NCH_T, TPC = cfg.NCH_T, cfg.TPC
    UC = cfg.OUT + 2                    # 18 used cols in table2
    MC = cfg.OUT + 1                    # 17 message cols

    consts = ctx.enter_context(tc.tile_pool(name="consts", bufs=1))
    iota = consts.tile([P, P], F32)
    nc.sync.dma_start(out=iota[:], in_=t["iota"][:, :])
    identb = consts.tile([P, P], BF16)
    nc.sync.dma_start(out=identb[:], in_=t["identb"][:, :])

    ipool = ctx.enter_context(tc.tile_pool(name="idx", bufs=2))
    gpool = ctx.enter_context(tc.tile_pool(name="g2", bufs=2))
    spool = ctx.enter_context(tc.tile_pool(name="s2", bufs=2))
    lpool = ctx.enter_context(tc.tile_pool(name="l2", bufs=2))
    mpool = ctx.enter_context(tc.tile_pool(name="m2", bufs=2))
    apsum = ctx.enter_context(tc.tile_pool(name="aps2", bufs=2,
                                           space="PSUM"))
    stps = ctx.enter_context(tc.tile_pool(name="stps2", bufs=2,
                                          space="PSUM"))
    seps = ctx.enter_context(tc.tile_pool(name="seps2", bufs=2,
                                          space="PSUM"))
    stsb = ctx.enter_context(tc.tile_pool(name="stsb2", bufs=3))
    opool = ctx.enter_context(tc.tile_pool(name="o", bufs=3))

    tab_pair = _ap(t["table2"][:, :], [[256, cfg.N_PAD // 2], [1, 256]])

    for ti in range(TPC):
        c0 = ti * NCH_T
        isrc = ipool.tile([P, NCH_T * 8], I16, tag="isrc")
        nc.sync.dma_start(out=isrc[:], in_=t["srcw"][
            :, ti * NCH_T * 8:(ti + 1) * NCH_T * 8])
        dloc = ipool.tile([P, NCH_T], F32, tag="dloc")
        nc.sync.dma_start(out=dloc[:], in_=t["dstloc_f"][:, c0:c0 + NCH_T])
        spar = ipool.tile([P, NCH_T], F32, tag="spar")
        nc.sync.dma_start(out=spar[:], in_=t["src_par"][:, c0:c0 + NCH_T])
        s2_tile = ipool.tile([P, 8], BF16, tag="s2tile")
        nc.sync.dma_start(out=s2_tile[:],
                          in_=t["s2slab"][ti * P:(ti + 1) * P, 0:8])

        G2 = gpool.tile([P, NCH_T, 256], BF16, tag="G2")
        nc.gpsimd.dma_gather(
            out_ap=G2[:], in_ap=tab_pair, idxs_ap=isrc[:],
            num_idxs=NCH_T * P, num_idxs_reg=NCH_T * P, elem_size=256,
            single_packet=False)

        # parity selects: x = odd + par*(even - odd)
        Rd = spool.tile([P, NCH_T, UC], F32, tag="Rd")
        nc.vector.tensor_tensor(out=Rd[:], in0=G2[:, :, 0:UC],
                                in1=G2[:, :, 128:128 + UC], op=OP.subtract)
        Rm = spool.tile([P, NCH_T, UC], F32, tag="Rm")
        nc.vector.tensor_tensor(
            out=Rm[:], in0=Rd[:],
            in1=_ap(spar[:], [spar[:].ap[0], [1, NCH_T], [0, UC]]),
            op=OP.mult)
        Ro = spool.tile([P, NCH_T, UC], F32, tag="Ro")
        nc.scalar.activation(Ro[:], G2[:, :, 128:128 + UC], AF.Copy)
        R = spool.tile([P, NCH_T, UC], F32, tag="R")
        nc.vector.tensor_tensor(out=R[:], in0=Rm[:], in1=Ro[:], op=OP.add)

        S = spool.tile([P, NCH_T, P], BF16, tag="S")
        nc.vector.tensor_tensor(
            out=S[:],
            in0=_ap(iota[:], [iota[:].ap[0], [0, NCH_T], [1, P]]),
            in1=_ap(dloc[:], [dloc[:].ap[0], [1, NCH_T], [0, P]]),
            op=OP.is_equal)

        # s2[dst] per edge via PE one-hot broadcast
        sesb = spool.tile([P, NCH_T, 1], F32, tag="sesb2")
        for k in range(NCH_T):
            st_ps = stps.tile([P, P], BF16, tag="st2")
            nc.tensor.transpose(out=st_ps[:], in_=S[:, k, :],
                                identity=identb[:])
            st_sb = stsb.tile([P, P], BF16, tag="stsb2")
            nc.vector.tensor_copy(st_sb[:], st_ps[:])
            se_ps = seps.tile([P, 1], F32, tag="se2")
            nc.tensor.matmul(out=se_ps[:], lhsT=st_sb[:],
                             rhs=s2_tile[:, 0:1], start=True, stop=True)
            nc.scalar.activation(sesb[:, k, :], se_ps[:], AF.Copy)

        u = lpool.tile([P, NCH_T, 1], F32, tag="u2")
        nc.vector.tensor_tensor(out=u[:], in0=sesb[:],
                                in1=R[:, :, cfg.OUT:MC], op=OP.add)
        a = lpool.tile([P, NCH_T, 1], F32, tag="a2")
        nc.vector.scalar_tensor_tensor(out=a[:], in0=u[:], scalar=cfg.NEG,
                                       in1=u[:], op0=OP.mult, op1=OP.max)
        ex = lpool.tile([P, NCH_T, 1], F32, tag="ex2")
        nc.scalar.activation(ex[:], a[:], AF.Exp)

        M = mpool.tile([P, NCH_T, MC], BF16, tag="M2")
        nc.scalar.activation(M[:, :, cfg.OUT:MC], ex[:], AF.Copy)
        nc.vector.tensor_tensor(
            out=M[:, :, 0:cfg.OUT],
            in0=R[:, :, 0:cfg.OUT],
            in1=_ap(ex[:], [ex[:].ap[0], [1, NCH_T], [0, cfg.OUT]]),
            op=OP.mult)

        agg = apsum.tile([P, MC], F32, tag="agg2")
        for k in range(NCH_T):
            nc.tensor.matmul(out=agg[:], lhsT=S[:, k, :], rhs=M[:, k, :],
                             start=(k == 0), stop=(k == NCH_T - 1))

        den = opool.tile([P, 1], F32, tag="den")
        nc.vector.tensor_scalar_add(den[:], agg[:, cfg.OUT:MC], 1e-20)
        rcp = opool.tile([P, 1], F32, tag="rcp")
        nc.vector.reciprocal(rcp[:], den[:])
        res = opool.tile([P, cfg.OUT + 1], F32, tag="res")
        nc.vector.tensor_tensor(
            out=res[:, 0:cfg.OUT], in0=agg[:, 0:cfg.OUT],
            in1=_ap(rcp[:], [rcp[:].ap[0], [0, cfg.OUT]]), op=OP.mult)
        # logits are O(0.3): exp() safe without max-subtraction; host
        # finishes log_softmax as h2 - log(sum_exp).
        pe = opool.tile([P, cfg.OUT], F32, tag="pe")
        nc.scalar.activation(pe[:], res[:, 0:cfg.OUT], AF.Exp,
                             accum_out=res[:, cfg.OUT:cfg.OUT + 1])
        nc.sync.dma_start(out=t["outp"][ti * P:(ti + 1) * P, :], in_=res[:])


def _decl_a1(nc, cfg):
    t = {}
    WCOLS = cfg.F + 2 * cfg.H

    def inp(name, shape, dt):
        t[name] = nc.dram_tensor(name, shape, dt, kind="ExternalInput").ap()

    inp("xTs", [P, cfg.NPC_PAD], F32)
    inp("wpack", [P, WCOLS], F32)
    t["t1slab"] = nc.dram_tensor("t1slab", [cfg.NPC_PAD, 256], BF16,
                                 kind="ExternalOutput").ap()
    t["sslab"] = nc.dram_tensor("sslab", [cfg.NPC_PAD, 8], BF16,
                                kind="ExternalOutput").ap()
    return t


def _decl_a2(nc, cfg):
    t = {}
    W2COLS = cfg.OUT + 2

    def inp(name, shape, dt):
        t[name] = nc.dram_tensor(name, shape, dt, kind="ExternalInput").ap()

    inp("table1", [cfg.N_PAD, 256], BF16)
    inp("sslab", [cfg.NPC_PAD, 8], BF16)
    inp("w2pack", [P, W2COLS], F32)
    inp("iota", [P, P], F32)
    inp("ident", [P, P], F32)
    inp("identb", [P, P], BF16)
    inp("srcw_lo", [P, cfg.TPC * cfg.NCHL_T * 8], I16)
    inp("srcw_hi", [P, cfg.TPC * cfg.NCHH_T * 8], I16)
    inp("dstloc_f", [P, cfg.NCH], F32)
    t["table2slab"] = nc.dram_tensor("table2slab", [cfg.NPC_PAD, W2COLS],
                                     F32, kind="ExternalOutput").ap()
    return t


def _decl_b(nc, cfg):
    t = {}

    def inp(name, shape, dt):
        t[name] = nc.dram_tensor(name, shape, dt, kind="ExternalInput").ap()

    inp("table2", [cfg.N_PAD, 128], BF16)
    inp("s2slab", [cfg.NPC_PAD, 8], BF16)
    inp("srcw", [P, cfg.NCH * 8], I16)
    inp("dstloc_f", [P, cfg.NCH], F32)
    inp("src_par", [P, cfg.NCH], F32)
    inp("iota", [P, P], F32)
    inp("identb", [P, P], BF16)
    t["outp"] = nc.dram_tensor("outp", [cfg.NPC_PAD, cfg.OUT + 1], F32,
                               kind="ExternalOutput").ap()
    return t


def _compile(build_fn, decl_fn, cfg):
    nc = bacc.Bacc("TRN2", target_bir_lowering=False, debug=False,
                   enable_asserts=False, num_devices=cfg.NCORES)
    t = decl_fn(nc, cfg)
    with tile.TileContext(nc) as tc:
        build_fn(tc, cfg, t)
    nc.compile()
    return nc


def _host_prep_weights(cfg, W1, att_src1, att_dst1, W2, att_src2, att_dst2):
    A_d1 = _blockdiag_att(np.asarray(att_dst1, np.float32), cfg.H, cfg.HID,
                          cfg.F)
    A_s1 = _blockdiag_att(np.asarray(att_src1, np.float32), cfg.H, cfg.HID,
                          cfg.F)
    W1T = np.asarray(W1, np.float32).T.copy()
    wpack = np.concatenate([W1T, W1T @ A_d1, W1T @ A_s1], axis=1)
    W2T = np.asarray(W2, np.float32).T.copy()
    a_d2 = np.asarray(att_dst2, np.float32).reshape(cfg.OUT, 1)
    a_s2 = np.asarray(att_src2, np.float32).reshape(cfg.OUT, 1)
    w2pack = np.concatenate([W2T, W2T @ a_d2, W2T @ a_s2], axis=1)
    return (np.ascontiguousarray(wpack, np.float32),
            np.ascontiguousarray(w2pack, np.float32))


_CACHE = {}


def _get_kernels(cfg):
    key = (cfg.N, cfg.E, cfg.NCORES, cfg.NCH_T, cfg.NCHL_T)
    if key not in _CACHE:
        nca1 = _compile(_build_a1, _decl_a1, cfg)
        nca2 = _compile(_build_a2, _decl_a2, cfg)
        ncb = _compile(_build_b, _decl_b, cfg)
        _CACHE[key] = (nca1, nca2, ncb)
    return _CACHE[key]


def run(cfg, inputs, runner=None):
    x = np.asarray(inputs["x"], np.float32)
    edge_index = np.asarray(inputs["edge_index"], np.int64)
    pc = _prep_graph(cfg, edge_index)
    wpack, w2pack = _host_prep_weights(
        cfg, inputs["W1"], inputs["att_src1"], inputs["att_dst1"],
        inputs["W2"], inputs["att_src2"], inputs["att_dst2"])

    xT = np.zeros((P, cfg.N_PAD), np.float32)
    xT[:, :cfg.N] = x.T
    iota = np.tile(np.arange(P, dtype=np.float32), (P, 1))
    ident = np.eye(P, dtype=np.float32)
    identb = np.eye(P, dtype=BF)

    nca1, nca2, ncb = _get_kernels(cfg)

    if runner is None:
        def runner(nc, in_maps):
            r = bass_utils.run_bass_kernel_spmd(
                nc, in_maps, core_ids=list(range(cfg.NCORES)))
            return r.results

    # --- A1: sharded node phase ---
    in_maps_a1 = []
    for c in range(cfg.NCORES):
        xTs = np.zeros((P, cfg.NPC_PAD), np.float32)
        n0 = c * cfg.NPC
        n1 = min(n0 + cfg.NPC_PAD, cfg.N_PAD)
        xTs[:, 0:n1 - n0] = xT[:, n0:n1]
        in_maps_a1.append(dict(xTs=xTs, wpack=wpack))
    res_a1 = runner(nca1, in_maps_a1)

    table1 = np.zeros((cfg.N_PAD, 256), BF)
    sslabs = []
    for c in range(cfg.NCORES):
        t1s = np.asarray(res_a1[c]["t1slab"])
        n0 = c * cfg.NPC
        n1 = min(n0 + cfg.NPC_PAD, cfg.N_PAD)
        take = min(cfg.NPC, n1 - n0)
        table1[n0:n0 + take] = t1s[:take]
        sslabs.append(np.asarray(res_a1[c]["sslab"]))

    # --- A2: layer-0 edge phase ---
    in_maps_a2 = []
    for c in range(cfg.NCORES):
        in_maps_a2.append(dict(
            table1=table1, sslab=sslabs[c], w2pack=w2pack, iota=iota,
            ident=ident, identb=identb,
            srcw_lo=pc["srcw_lo"][c], srcw_hi=pc["srcw_hi"][c],
            dstloc_f=pc["dstloc_f"][c]))
    res_a2 = runner(nca2, in_maps_a2)

    table2 = np.zeros((cfg.N_PAD, 128), BF)
    s2slabs = []
    for c in range(cfg.NCORES):
        slab = np.asarray(res_a2[c]["table2slab"], np.float32)
        table2[c * cfg.NPC:(c + 1) * cfg.NPC, 0:cfg.OUT + 2] = \
            slab[:cfg.NPC].astype(BF)
        s2s = np.zeros((cfg.NPC_PAD, 8), BF)
        s2s[:, 0] = slab[:, cfg.OUT + 1].astype(BF)
        s2slabs.append(s2s)

    # --- B: layer-1 edge phase ---
    in_maps_b = []
    for c in range(cfg.NCORES):
        in_maps_b.append(dict(
            table2=table2, s2slab=s2slabs[c], srcw=pc["srcw"][c],
            dstloc_f=pc["dstloc_f"][c], src_par=pc["src_par"][c],
            iota=iota, identb=identb))
    res_b = runner(ncb, in_maps_b)

    out = np.zeros((cfg.N, cfg.OUT), np.float32)
    for c in range(cfg.NCORES):
        ob = np.asarray(res_b[c]["outp"], np.float32)[:cfg.NPC]
        out[c * cfg.NPC:(c + 1) * cfg.NPC] = \
            ob[:, 0:cfg.OUT] - np.log(ob[:, cfg.OUT:cfg.OUT + 1])
    return out


def kernel(**inputs):
    cfg = Cfg(N=50000, E=1600000, ncores=8)
    return run(cfg, inputs)


# revision 6
# speedup vs baseline: 1.0315x; 1.0315x over previous
"""
2-layer GAT on Trainium2 (8 NeuronCores, SPMD via bass/Tile).

Sharding: destination nodes are block-sharded across the 8 cores (6250
nodes each).  All per-edge work runs on the core owning the edge's dst.

Pipeline = three bass kernels with host gathers between them:
  A1: node phase, sharded — core c computes h = x@W1pack for its own
      ~6272 nodes only, writes t1slab [h|d1] (bf16) + sslab s1 (bf16).
      Host assembles the full table1 from the 8 slabs.
  A2: layer-0 edge phase.  Per-edge h|d1 rows come from one
      dma_gather per edge (512 B rows, lo/hi split for int16 idx).
      s1[dst] is NOT gathered: dst rows of a tile are contiguous, so a
      [128,8] s-tile is DMA'd and broadcast per edge on the PE via
      S_T = transpose(S) followed by se = S_T.T @ s_tile (exact: S is
      one-hot).  Aggregation per chunk of 128 edges: S.T @ [feat*ex|ex]
      accumulated in PSUM; then normalize, ELU, W2 matmul -> table2slab.
  B:  layer-1 edge phase.  feat2|d2[src] via one pair-row dma_gather
      per edge (int16 idx = src>>1 + parity select).  s2[dst] via the
      same PE one-hot broadcast trick.  log_softmax tail.

Softmax max-subtraction is skipped: logits are O(0.3) so exp() is
stable, and softmax is shift-invariant.
"""

import os
import sys
from contextlib import ExitStack

import numpy as np
import ml_dtypes

for _p in ("/opt/trn_rl_repo",):
    if os.path.isdir(_p) and _p not in sys.path:
        sys.path.insert(0, _p)

import concourse.bass as bass
import concourse.bacc as bacc
import concourse.tile as tile
from concourse import mybir
from concourse import bass_utils
from concourse._compat import with_exitstack

F32 = mybir.dt.float32
BF16 = mybir.dt.bfloat16
I32 = mybir.dt.int32
I16 = mybir.dt.int16
AF = mybir.ActivationFunctionType
OP = mybir.AluOpType
P = 128
BF = ml_dtypes.bfloat16


class Cfg:
    def __init__(self, N, E, ncores, split=32768, neg=0.2, in_ch=128,
                 f=128, heads=8, hid=16, out=16):
        self.N = N
        self.E = E
        self.NCORES = ncores
        self.SPLIT = split
        self.NEG = neg
        self.IN = in_ch
        self.F = f
        self.H = heads
        self.HID = hid
        self.OUT = out
        assert N % ncores == 0
        self.NPC = N // ncores
        self.TPC = (self.NPC + P - 1) // P
        self.NPC_PAD = self.TPC * P
        self.NTILES = ncores * self.TPC
        self.N_PAD = self.NTILES * P
        self.NCHL_T = None
        self.NCHH_T = None
        self.NCH_T = None
        self.NCH = None


def _wrap16(vals):
    """[n] slot-ordered values -> [128, n//16] int16 wrapped layout."""
    n = vals.shape[0]
    assert n % 16 == 0
    w = vals.reshape(-1, 16).T.astype(np.int16)      # [16, n//16]
    return np.ascontiguousarray(np.tile(w, (8, 1)))  # [128, n//16]


def _prep_graph(cfg, edge_index):
    N, NPC, SPL = cfg.N, cfg.NPC, cfg.SPLIT
    src = np.concatenate([edge_index[0], np.arange(N, dtype=np.int64)])
    dst = np.concatenate([edge_index[1], np.arange(N, dtype=np.int64)])
    core = dst // NPC
    ld = dst - core * NPC
    tile_id = ld // P
    dstloc = ld % P
    hi = (src >= SPL).astype(np.int64)
    order = np.lexsort((src, hi, tile_id, core))
    src, dst, core, tile_id, dstloc, hi = (a[order] for a in
                                           (src, dst, core, tile_id,
                                            dstloc, hi))
    keyf = (core * cfg.TPC + tile_id) * 2 + hi
    cntf = np.bincount(keyf, minlength=cfg.NCORES * cfg.TPC * 2)
    cnt_lo = cntf[0::2].reshape(cfg.NCORES, cfg.TPC)
    cnt_hi = cntf[1::2].reshape(cfg.NCORES, cfg.TPC)
    cfg.NCHL_T = max(1, int(np.max((cnt_lo + P - 1) // P)))
    cfg.NCHH_T = max(1, int(np.max((cnt_hi + P - 1) // P)))
    cfg.NCH_T = cfg.NCHL_T + cfg.NCHH_T
    cfg.NCH = cfg.NCH_T * cfg.TPC
    starts = np.concatenate([[0], np.cumsum(cntf)])

    pc = dict(srcw_lo=[], srcw_hi=[], srcw=[], dstloc_f=[], src_par=[])
    for c in range(cfg.NCORES):
        ns_lo = cfg.TPC * cfg.NCHL_T * P
        ns_hi = cfg.TPC * cfg.NCHH_T * P
        ns = cfg.NCH * P
        v_srclo = np.zeros(ns_lo, np.int64)
        v_srchi = np.zeros(ns_hi, np.int64)
        v_src = np.zeros(ns, np.int64)          # src>>1 per slot
        v_dstloc = np.full(ns, -1.0, np.float32)
        v_spar = np.ones(ns, np.float32)
        for t in range(cfg.TPC):
            for h in (0, 1):
                k = ((c * cfg.TPC + t) * 2 + h)
                n = int(cntf[k])
                if n == 0:
                    continue
                sl = slice(starts[k], starts[k] + n)
                e_src = src[sl]
                e_dl = dstloc[sl]
                pos = np.arange(n)
                if h == 0:
                    v_srclo[t * cfg.NCHL_T * P + pos] = e_src
                    ch = t * cfg.NCH_T + pos // P
                else:
                    v_srchi[t * cfg.NCHH_T * P + pos] = e_src - SPL
                    ch = t * cfg.NCH_T + cfg.NCHL_T + pos // P
                slot = ch * P + pos % P
                v_src[slot] = e_src >> 1
                v_dstloc[slot] = e_dl
                v_spar[slot] = 1.0 - (e_src & 1)
        pc["srcw_lo"].append(_wrap16(v_srclo))
        pc["srcw_hi"].append(_wrap16(v_srchi))
        pc["srcw"].append(_wrap16(v_src))
        # slot arrays in [128, NCH] layout: slot = ch*128 + p -> [p, ch]
        pc["dstloc_f"].append(
            np.ascontiguousarray(v_dstloc.reshape(cfg.NCH, P).T))
        pc["src_par"].append(np.ascontiguousarray(
            v_spar.reshape(cfg.NCH, P).T))
    return pc


def _blockdiag_att(att, heads, hid, f):
    A = np.zeros((f, heads), dtype=np.float32)
    for h in range(heads):
        A[h * hid:(h + 1) * hid, h] = att[0, h]
    return A


def _ap(base, ap_list, off_extra=0):
    return bass.AP(tensor=base.tensor, offset=base.offset + off_extra,
                   ap=ap_list)


@with_exitstack
def _build_a1(ctx, tc, cfg, t):
    """Sharded node phase: this core's TPC tiles only."""
    nc = tc.nc
    WCOLS = cfg.F + 2 * cfg.H             # 144 matmul out cols
    TCOLS = cfg.F + cfg.H                 # 136 table1 used cols

    consts = ctx.enter_context(tc.tile_pool(name="consts", bufs=1))
    wpack = consts.tile([P, WCOLS], F32)
    nc.sync.dma_start(out=wpack[:], in_=t["wpack"][:, :])

    NT = cfg.TPC
    BLK = 8
    xpool = ctx.enter_context(tc.tile_pool(name="xt", bufs=2))
    npsum = ctx.enter_context(tc.tile_pool(name="npsum", bufs=2,
                                           space="PSUM"))
    nstage = ctx.enter_context(tc.tile_pool(name="nstage", bufs=3))
    for blk in range((NT + BLK - 1) // BLK):
        nt0 = blk * BLK
        nt1 = min(nt0 + BLK, NT)
        xt = xpool.tile([P, BLK * P], F32, tag="xt")
        nc.sync.dma_start(out=xt[:, 0:(nt1 - nt0) * P],
                          in_=t["xTs"][:, nt0 * P:nt1 * P])
        for j in range(nt1 - nt0):
            nt = nt0 + j
            pt = npsum.tile([P, WCOLS], F32, tag="npt")
            nc.tensor.matmul(out=pt[:], lhsT=xt[:, j * P:(j + 1) * P],
                             rhs=wpack[:], start=True, stop=True)
            s1 = nstage.tile([P, TCOLS], BF16, tag="s1")
            nc.scalar.activation(s1[:], pt[:, 0:TCOLS], AF.Copy)
            ss = nstage.tile([P, cfg.H], BF16, tag="ss")
            nc.scalar.activation(ss[:], pt[:, TCOLS:WCOLS], AF.Copy)
            nc.sync.dma_start(
                out=t["t1slab"][nt * P:(nt + 1) * P, 0:TCOLS], in_=s1[:])
            nc.sync.dma_start(
                out=t["sslab"][nt * P:(nt + 1) * P, 0:cfg.H], in_=ss[:])


@with_exitstack
def _build_a2(ctx, tc, cfg, t):
    nc = tc.nc
    NCH_T, NCHL_T, NCHH_T, TPC = cfg.NCH_T, cfg.NCHL_T, cfg.NCHH_T, cfg.TPC
    MCOLS = cfg.F + cfg.H                 # 136 message cols
    ROW1 = 256                            # table1 row elems (bf16)
    W2COLS = cfg.OUT + 2

    consts = ctx.enter_context(tc.tile_pool(name="consts", bufs=1))
    w2pack = consts.tile([P, W2COLS], F32)
    nc.sync.dma_start(out=w2pack[:], in_=t["w2pack"][:, :])
    iota = consts.tile([P, P], F32)
    nc.sync.dma_start(out=iota[:], in_=t["iota"][:, :])
    ident = consts.tile([P, P], F32)
    nc.sync.dma_start(out=ident[:], in_=t["ident"][:, :])
    identb = consts.tile([P, P], BF16)
    nc.sync.dma_start(out=identb[:], in_=t["identb"][:, :])

    gpool = ctx.enter_context(tc.tile_pool(name="g", bufs=3))
    ipool = ctx.enter_context(tc.tile_pool(name="idx", bufs=3))
    spool = ctx.enter_context(tc.tile_pool(name="sel", bufs=3))
    mpool = ctx.enter_context(tc.tile_pool(name="msg", bufs=2))
    lpool = ctx.enter_context(tc.tile_pool(name="logit", bufs=2))
    apsum = ctx.enter_context(tc.tile_pool(name="apsum", bufs=2,
                                           space="PSUM"))
    stpsum = ctx.enter_context(tc.tile_pool(name="stpsum", bufs=2,
                                            space="PSUM"))
    sepsum = ctx.enter_context(tc.tile_pool(name="sepsum", bufs=2,
                                            space="PSUM"))
    tpsum = ctx.enter_context(tc.tile_pool(name="tpsum", bufs=1,
                                           space="PSUM"))
    t2psum = ctx.enter_context(tc.tile_pool(name="t2psum", bufs=1,
                                            space="PSUM"))
    stsb = ctx.enter_context(tc.tile_pool(name="stsb", bufs=3))
    hpool = ctx.enter_context(tc.tile_pool(name="h1", bufs=2))

    tab_hi = t["table1"][cfg.SPLIT:cfg.N_PAD, :]

    for ti in range(TPC):
        c0 = ti * NCH_T
        il = ipool.tile([P, NCHL_T * 8], I16, tag="il")
        nc.sync.dma_start(out=il[:], in_=t["srcw_lo"][
            :, ti * NCHL_T * 8:(ti + 1) * NCHL_T * 8])
        ih = ipool.tile([P, NCHH_T * 8], I16, tag="ih")
        nc.sync.dma_start(out=ih[:], in_=t["srcw_hi"][
            :, ti * NCHH_T * 8:(ti + 1) * NCHH_T * 8])
        dloc = ipool.tile([P, NCH_T], F32, tag="dloc")
        nc.sync.dma_start(out=dloc[:], in_=t["dstloc_f"][:, c0:c0 + NCH_T])
        s_tile = ipool.tile([P, cfg.H], BF16, tag="stile")
        nc.sync.dma_start(out=s_tile[:],
                          in_=t["sslab"][ti * P:(ti + 1) * P, 0:cfg.H])

        # gathers (h|d1 by src)
        G = gpool.tile([P, NCH_T, ROW1], BF16, tag="G")
        nc.gpsimd.dma_gather(
            out_ap=G[:, 0:NCHL_T, :], in_ap=t["table1"][:, :],
            idxs_ap=il[:], num_idxs=NCHL_T * P, num_idxs_reg=NCHL_T * P,
            elem_size=ROW1, single_packet=False)
        nc.gpsimd.dma_gather(
            out_ap=G[:, NCHL_T:NCH_T, :], in_ap=tab_hi,
            idxs_ap=ih[:], num_idxs=NCHH_T * P, num_idxs_reg=NCHH_T * P,
            elem_size=ROW1, single_packet=False)

        # one-hot S
        S = spool.tile([P, NCH_T, P], BF16, tag="S")
        nc.vector.tensor_tensor(
            out=S[:],
            in0=_ap(iota[:], [iota[:].ap[0], [0, NCH_T], [1, P]]),
            in1=_ap(dloc[:], [dloc[:].ap[0], [1, NCH_T], [0, P]]),
            op=OP.is_equal)

        # s1[dst] per edge via PE: se = (S_k)^T.T @ s_tile
        sesb = spool.tile([P, NCH_T, cfg.H], F32, tag="sesb")
        for k in range(NCH_T):
            st_ps = stpsum.tile([P, P], BF16, tag="st")
            nc.tensor.transpose(out=st_ps[:], in_=S[:, k, :],
                                identity=identb[:])
            st_sb = stsb.tile([P, P], BF16, tag="stsb")
            nc.scalar.activation(st_sb[:], st_ps[:], AF.Copy)
            se_ps = sepsum.tile([P, cfg.H], F32, tag="se")
            nc.tensor.matmul(out=se_ps[:], lhsT=st_sb[:], rhs=s_tile[:],
                             start=True, stop=True)
            nc.scalar.activation(sesb[:, k, :], se_ps[:], AF.Copy)

        # logits -> ex (bf16)
        dcp = lpool.tile([P, NCH_T, cfg.H], F32, tag="dcp")
        nc.scalar.activation(dcp[:], G[:, :, cfg.F:cfg.F + cfg.H], AF.Copy)
        u = lpool.tile([P, NCH_T, cfg.H], F32, tag="u")
        nc.vector.tensor_tensor(out=u[:], in0=sesb[:], in1=dcp[:], op=OP.add)
        a = lpool.tile([P, NCH_T, cfg.H], F32, tag="a")
        nc.vector.scalar_tensor_tensor(out=a[:], in0=u[:], scalar=cfg.NEG,
                                       in1=u[:], op0=OP.mult, op1=OP.max)
        ex = lpool.tile([P, NCH_T, cfg.H], BF16, tag="ex")
        nc.scalar.activation(ex[:], a[:], AF.Exp)

        # Msg = [feat * ex | ex]
        M = mpool.tile([P, NCH_T, MCOLS], BF16, tag="M")
        nc.scalar.activation(M[:, :, cfg.F:MCOLS], ex[:], AF.Copy)
        nc.vector.tensor_tensor(
            out=_ap(M[:], [M[:].ap[0], [MCOLS, NCH_T], [cfg.HID, cfg.H],
                           [1, cfg.HID]]),
            in0=_ap(G[:], [G[:].ap[0], [ROW1, NCH_T], [cfg.HID, cfg.H],
                           [1, cfg.HID]]),
            in1=_ap(ex[:], [ex[:].ap[0], [cfg.H, NCH_T], [1, cfg.H],
                            [0, cfg.HID]]),
            op=OP.mult)

        # aggregate
        agg = apsum.tile([P, MCOLS], F32, tag="agg")
        for k in range(NCH_T):
            nc.tensor.matmul(out=agg[:], lhsT=S[:, k, :], rhs=M[:, k, :],
                             start=(k == 0), stop=(k == NCH_T - 1))

        # normalize + elu + feat2/d2/s2 slab
        den = hpool.tile([P, cfg.H], F32, tag="den")
        nc.vector.tensor_scalar_add(den[:], agg[:, cfg.F:MCOLS], 1e-20)
        rcp = hpool.tile([P, cfg.H], F32, tag="rcp")
        nc.vector.reciprocal(rcp[:], den[:])
        h1 = hpool.tile([P, cfg.F], F32, tag="h1")
        nc.vector.tensor_tensor(
            out=_ap(h1[:], [h1[:].ap[0], [cfg.HID, cfg.H], [1, cfg.HID]]),
            in0=_ap(agg[:], [agg[:].ap[0], [cfg.HID, cfg.H], [1, cfg.HID]]),
            in1=_ap(rcp[:], [rcp[:].ap[0], [1, cfg.H], [0, cfg.HID]]),
            op=OP.mult)
        neg = hpool.tile([P, cfg.F], F32, tag="neg")
        nc.vector.tensor_scalar_min(neg[:], h1[:], 0.0)
        pos = hpool.tile([P, cfg.F], F32, tag="pos")
        nc.vector.tensor_scalar_max(pos[:], h1[:], 0.0)
        een = hpool.tile([P, cfg.F], F32, tag="een")
        nc.scalar.activation(een[:], neg[:], AF.Exp)
        elu = hpool.tile([P, cfg.F], F32, tag="elu")
        nc.vector.scalar_tensor_tensor(out=elu[:], in0=een[:], scalar=-1.0,
                                       in1=pos[:], op0=OP.add, op1=OP.add)
        eT_ps = tpsum.tile([P, P], F32, tag="eT")
        nc.tensor.transpose(out=eT_ps[:], in_=elu[:], identity=ident[:])
        eT = hpool.tile([P, P], F32, tag="eTs")
        nc.scalar.activation(eT[:], eT_ps[:], AF.Copy)
        t2 = t2psum.tile([P, W2COLS], F32, tag="t2")
        nc.tensor.matmul(out=t2[:], lhsT=eT[:], rhs=w2pack[:],
                         start=True, stop=True)
        t2s = hpool.tile([P, W2COLS], F32, tag="t2s")
        nc.scalar.activation(t2s[:], t2[:], AF.Copy)
        nc.sync.dma_start(out=t["table2slab"][ti * P:(ti + 1) * P, :],
                          in_=t2s[:])


@with_exitstack
def _build_b(ctx, tc, cfg, t):
    nc = tc.nc
    NCH_T, TPC = cfg.NCH_T, cfg.TPC
    UC = cfg.OUT + 2                    # 18 used cols in table2
    MC = cfg.OUT + 1                    # 17 message cols

    consts = ctx.enter_context(tc.tile_pool(name="consts", bufs=1))
    iota = consts.tile([P, P], F32)
    nc.sync.dma_start(out=iota[:], in_=t["iota"][:, :])
    identb = consts.tile([P, P], BF16)
    nc.sync.dma_start(out=identb[:], in_=t["identb"][:, :])

    ipool = ctx.enter_context(tc.tile_pool(name="idx", bufs=3))
    gpool = ctx.enter_context(tc.tile_pool(name="g2", bufs=3))
    spool = ctx.enter_context(tc.tile_pool(name="s2", bufs=3))
    lpool = ctx.enter_context(tc.tile_pool(name="l2", bufs=2))
    mpool = ctx.enter_context(tc.tile_pool(name="m2", bufs=2))
    apsum = ctx.enter_context(tc.tile_pool(name="aps2", bufs=2,
                                           space="PSUM"))
    stps = ctx.enter_context(tc.tile_pool(name="stps2", bufs=2,
                                          space="PSUM"))
    seps = ctx.enter_context(tc.tile_pool(name="seps2", bufs=2,
                                          space="PSUM"))
    stsb = ctx.enter_context(tc.tile_pool(name="stsb2", bufs=3))
    opool = ctx.enter_context(tc.tile_pool(name="o", bufs=3))

    tab_pair = _ap(t["table2"][:, :], [[256, cfg.N_PAD // 2], [1, 256]])

    for ti in range(TPC):
        c0 = ti * NCH_T
        isrc = ipool.tile([P, NCH_T * 8], I16, tag="isrc")
        nc.sync.dma_start(out=isrc[:], in_=t["srcw"][
            :, ti * NCH_T * 8:(ti + 1) * NCH_T * 8])
        dloc = ipool.tile([P, NCH_T], F32, tag="dloc")
        nc.sync.dma_start(out=dloc[:], in_=t["dstloc_f"][:, c0:c0 + NCH_T])
        spar = ipool.tile([P, NCH_T], F32, tag="spar")
        nc.sync.dma_start(out=spar[:], in_=t["src_par"][:, c0:c0 + NCH_T])
        s2_tile = ipool.tile([P, 8], BF16, tag="s2tile")
        nc.sync.dma_start(out=s2_tile[:],
                          in_=t["s2slab"][ti * P:(ti + 1) * P, 0:8])

        G2 = gpool.tile([P, NCH_T, 256], BF16, tag="G2")
        nc.gpsimd.dma_gather(
            out_ap=G2[:], in_ap=tab_pair, idxs_ap=isrc[:],
            num_idxs=NCH_T * P, num_idxs_reg=NCH_T * P, elem_size=256,
            single_packet=False)

        # parity selects: x = odd + par*(even - odd)
        Rd = spool.tile([P, NCH_T, UC], F32, tag="Rd")
        nc.vector.tensor_tensor(out=Rd[:], in0=G2[:, :, 0:UC],
                                in1=G2[:, :, 128:128 + UC], op=OP.subtract)
        Rm = spool.tile([P, NCH_T, UC], F32, tag="Rm")
        nc.vector.tensor_tensor(
            out=Rm[:], in0=Rd[:],
            in1=_ap(spar[:], [spar[:].ap[0], [1, NCH_T], [0, UC]]),
            op=OP.mult)
        Ro = spool.tile([P, NCH_T, UC], F32, tag="Ro")
        nc.scalar.activation(Ro[:], G2[:, :, 128:128 + UC], AF.Copy)
        R = spool.tile([P, NCH_T, UC], F32, tag="R")
        nc.vector.tensor_tensor(out=R[:], in0=Rm[:], in1=Ro[:], op=OP.add)

        S = spool.tile([P, NCH_T, P], BF16, tag="S")
        nc.vector.tensor_tensor(
            out=S[:],
            in0=_ap(iota[:], [iota[:].ap[0], [0, NCH_T], [1, P]]),
            in1=_ap(dloc[:], [dloc[:].ap[0], [1, NCH_T], [0, P]]),
            op=OP.is_equal)

        # s2[dst] per edge via PE one-hot broadcast
        sesb = spool.tile([P, NCH_T, 1], F32, tag="sesb2")
        for k in range(NCH_T):
            st_ps = stps.tile([P, P], BF16, tag="st2")
            nc.tensor.transpose(out=st_ps[:], in_=S[:, k, :],
                                identity=identb[:])
            st_sb = stsb.tile([P, P], BF16, tag="stsb2")
            nc.vector.tensor_copy(st_sb[:], st_ps[:])
            se_ps = seps.tile([P, 1], F32, tag="se2")
            nc.tensor.matmul(out=se_ps[:], lhsT=st_sb[:],
                             rhs=s2_tile[:, 0:1], start=True, stop=True)
            nc.scalar.activation(sesb[:, k, :], se_ps[:], AF.Copy)

        u = lpool.tile([P, NCH_T, 1], F32, tag="u2")
        nc.vector.tensor_tensor(out=u[:], in0=sesb[:],
                                in1=R[:, :, cfg.OUT:MC], op=OP.add)
        a = lpool.tile([P, NCH_T, 1], F32, tag="a2")
        nc.vector.scalar_tensor_tensor(out=a[:], in0=u[:], scalar=cfg.NEG,
                                       in1=u[:], op0=OP.mult, op1=OP.max)
        ex = lpool.tile([P, NCH_T, 1], F32, tag="ex2")
        nc.scalar.activation(ex[:], a[:], AF.Exp)

        M = mpool.tile([P, NCH_T, MC], BF16, tag="M2")
        nc.scalar.activation(M[:, :, cfg.OUT:MC], ex[:], AF.Copy)
        nc.vector.tensor_tensor(
            out=M[:, :, 0:cfg.OUT],
            in0=R[:, :, 0:cfg.OUT],
            in1=_ap(ex[:], [ex[:].ap[0], [1, NCH_T], [0, cfg.OUT]]),
            op=OP.mult)

        agg = apsum.tile([P, MC], F32, tag="agg2")
        for k in range(NCH_T):
            nc.tensor.matmul(out=agg[:], lhsT=S[:, k, :], rhs=M[:, k, :],
                             start=(k == 0), stop=(k == NCH_T - 1))

        den = opool.tile([P, 1], F32, tag="den")
        nc.vector.tensor_scalar_add(den[:], agg[:, cfg.OUT:MC], 1e-20)
        rcp = opool.tile([P, 1], F32, tag="rcp")
        nc.vector.reciprocal(rcp[:], den[:])
        res = opool.tile([P, cfg.OUT + 1], F32, tag="res")
        nc.vector.tensor_tensor(
            out=res[:, 0:cfg.OUT], in0=agg[:, 0:cfg.OUT],
            in1=_ap(rcp[:], [rcp[:].ap[0], [0, cfg.OUT]]), op=OP.mult)
        # logits are O(0.3): exp() safe without max-subtraction; host
        # finishes log_softmax as h2 - log(sum_exp).
        pe = opool.tile([P, cfg.OUT], F32, tag="pe")
        nc.scalar.activation(pe[:], res[:, 0:cfg.OUT], AF.Exp,
                             accum_out=res[:, cfg.OUT:cfg.OUT + 1])
        nc.sync.dma_start(out=t["outp"][ti * P:(ti + 1) * P, :], in_=res[:])


def _decl_a1(nc, cfg):
    t = {}
    WCOLS = cfg.F + 2 * cfg.H

    def inp(name, shape, dt):
        t[name] = nc.dram_tensor(name, shape, dt, kind="ExternalInput").ap()

    inp("xTs", [P, cfg.NPC_PAD], F32)
    inp("wpack", [P, WCOLS], F32)
    t["t1slab"] = nc.dram_tensor("t1slab", [cfg.NPC_PAD, 256], BF16,
                                 kind="ExternalOutput").ap()
    t["sslab"] = nc.dram_tensor("sslab", [cfg.NPC_PAD, 8], BF16,
                                kind="ExternalOutput").ap()
    return t


def _decl_a2(nc, cfg):
    t = {}
    W2COLS = cfg.OUT + 2

    def inp(name, shape, dt):
        t[name] = nc.dram_tensor(name, shape, dt, kind="ExternalInput").ap()

    inp("table1", [cfg.N_PAD, 256], BF16)
    inp("sslab", [cfg.NPC_PAD, 8], BF16)
    inp("w2pack", [P, W2COLS], F32)
    inp("iota", [P, P], F32)
    inp("ident", [P, P], F32)
    inp("identb", [P, P], BF16)
    inp("srcw_lo", [P, cfg.TPC * cfg.NCHL_T * 8], I16)
    inp("srcw_hi", [P, cfg.TPC * cfg.NCHH_T * 8], I16)
    inp("dstloc_f", [P, cfg.NCH], F32)
    t["table2slab"] = nc.dram_tensor("table2slab", [cfg.NPC_PAD, W2COLS],
                                     F32, kind="ExternalOutput").ap()
    return t


def _decl_b(nc, cfg):
    t = {}

    def inp(name, shape, dt):
        t[name] = nc.dram_tensor(name, shape, dt, kind="ExternalInput").ap()

    inp("table2", [cfg.N_PAD, 128], BF16)
    inp("s2slab", [cfg.NPC_PAD, 8], BF16)
    inp("srcw", [P, cfg.NCH * 8], I16)
    inp("dstloc_f", [P, cfg.NCH], F32)
    inp("src_par", [P, cfg.NCH], F32)
    inp("iota", [P, P], F32)
    inp("identb", [P, P], BF16)
    t["outp"] = nc.dram_tensor("outp", [cfg.NPC_PAD, cfg.OUT + 1], F32,
                               kind="ExternalOutput").ap()
    return t


def _compile(build_fn, decl_fn, cfg):
    nc = bacc.Bacc("TRN2", target_bir_lowering=False, debug=False,
                   enable_asserts=False, num_devices=cfg.NCORES)
    t = decl_fn(nc, cfg)
    with tile.TileContext(nc) as tc:
        build_fn(tc, cfg, t)
    nc.compile()
    return nc


def _host_prep_weights(cfg, W1, att_src1, att_dst1, W2, att_src2, att_dst2):
    A_d1 = _blockdiag_att(np.asarray(att_dst1, np.float32), cfg.H, cfg.HID,
                          cfg.F)
    A_s1 = _blockdiag_att(np.asarray(att_src1, np.float32), cfg.H, cfg.HID,
                          cfg.F)
    W1T = np.asarray(W1, np.float32).T.copy()
    wpack = np.concatenate([W1T, W1T @ A_d1, W1T @ A_s1], axis=1)
    W2T = np.asarray(W2, np.float32).T.copy()
    a_d2 = np.asarray(att_dst2, np.float32).reshape(cfg.OUT, 1)
    a_s2 = np.asarray(att_src2, np.float32).reshape(cfg.OUT, 1)
    w2pack = np.concatenate([W2T, W2T @ a_d2, W2T @ a_s2], axis=1)
    return (np.ascontiguousarray(wpack, np.float32),
            np.ascontiguousarray(w2pack, np.float32))


_CACHE = {}


def _get_kernels(cfg):
    key = (cfg.N, cfg.E, cfg.NCORES, cfg.NCH_T, cfg.NCHL_T)
    if key not in _CACHE:
        nca1 = _compile(_build_a1, _decl_a1, cfg)
        nca2 = _compile(_build_a2, _decl_a2, cfg)
        ncb = _compile(_build_b, _decl_b, cfg)
        _CACHE[key] = (nca1, nca2, ncb)
    return _CACHE[key]


def run(cfg, inputs, runner=None):
    x = np.asarray(inputs["x"], np.float32)
    edge_index = np.asarray(inputs["edge_index"], np.int64)
    pc = _prep_graph(cfg, edge_index)
    wpack, w2pack = _host_prep_weights(
        cfg, inputs["W1"], inputs["att_src1"], inputs["att_dst1"],
        inputs["W2"], inputs["att_src2"], inputs["att_dst2"])

    xT = np.zeros((P, cfg.N_PAD), np.float32)
    xT[:, :cfg.N] = x.T
    iota = np.tile(np.arange(P, dtype=np.float32), (P, 1))
    ident = np.eye(P, dtype=np.float32)
    identb = np.eye(P, dtype=BF)

    nca1, nca2, ncb = _get_kernels(cfg)

    if runner is None:
        def runner(nc, in_maps):
            r = bass_utils.run_bass_kernel_spmd(
                nc, in_maps, core_ids=list(range(cfg.NCORES)))
            return r.results

    # --- A1: sharded node phase ---
    in_maps_a1 = []
    for c in range(cfg.NCORES):
        xTs = np.zeros((P, cfg.NPC_PAD), np.float32)
        n0 = c * cfg.NPC
        n1 = min(n0 + cfg.NPC_PAD, cfg.N_PAD)
        xTs[:, 0:n1 - n0] = xT[:, n0:n1]
        in_maps_a1.append(dict(xTs=xTs, wpack=wpack))
    res_a1 = runner(nca1, in_maps_a1)

    table1 = np.zeros((cfg.N_PAD, 256), BF)
    sslabs = []
    for c in range(cfg.NCORES):
        t1s = np.asarray(res_a1[c]["t1slab"])
        n0 = c * cfg.NPC
        n1 = min(n0 + cfg.NPC_PAD, cfg.N_PAD)
        take = min(cfg.NPC, n1 - n0)
        table1[n0:n0 + take] = t1s[:take]
        sslabs.append(np.asarray(res_a1[c]["sslab"]))

    # --- A2: layer-0 edge phase ---
    in_maps_a2 = []
    for c in range(cfg.NCORES):
        in_maps_a2.append(dict(
            table1=table1, sslab=sslabs[c], w2pack=w2pack, iota=iota,
            ident=ident, identb=identb,
            srcw_lo=pc["srcw_lo"][c], srcw_hi=pc["srcw_hi"][c],
            dstloc_f=pc["dstloc_f"][c]))
    res_a2 = runner(nca2, in_maps_a2)

    table2 = np.zeros((cfg.N_PAD, 128), BF)
    s2slabs = []
    for c in range(cfg.NCORES):
        slab = np.asarray(res_a2[c]["table2slab"], np.float32)
        table2[c * cfg.NPC:(c + 1) * cfg.NPC, 0:cfg.OUT + 2] = \
            slab[:cfg.NPC].astype(BF)
        s2s = np.zeros((cfg.NPC_PAD, 8), BF)
        s2s[:, 0] = slab[:, cfg.OUT + 1].astype(BF)
        s2slabs.append(s2s)

    # --- B: layer-1 edge phase ---
    in_maps_b = []
    for c in range(cfg.NCORES):
        in_maps_b.append(dict(
            table2=table2, s2slab=s2slabs[c], srcw=pc["srcw"][c],
            dstloc_f=pc["dstloc_f"][c], src_par=pc["src_par"][c],
            iota=iota, identb=identb))
    res_b = runner(ncb, in_maps_b)

    out = np.zeros((cfg.N, cfg.OUT), np.float32)
    for c in range(cfg.NCORES):
        ob = np.asarray(res_b[c]["outp"], np.float32)[:cfg.NPC]
        out[c * cfg.NPC:(c + 1) * cfg.NPC] = \
            ob[:, 0:cfg.OUT] - np.log(ob[:, cfg.OUT:cfg.OUT + 1])
    return out


def kernel(**inputs):
    cfg = Cfg(N=50000, E=1600000, ncores=8)
    return run(cfg, inputs)


# revision 7
# speedup vs baseline: 1.0996x; 1.0660x over previous
"""
2-layer GAT on Trainium2 (8 NeuronCores, SPMD via bass/Tile).

Sharding: destination nodes are block-sharded across the 8 cores (6250
nodes each).  All per-edge work runs on the core owning the edge's dst.

Pipeline = three bass kernels with host gathers between them:
  A1: node phase, sharded — core c computes h = x@W1pack for its own
      ~6272 nodes only, writes t1slab [h|d1] (bf16) + sslab s1 (bf16).
      Host assembles the full table1 from the 8 slabs.
  A2: layer-0 edge phase.  Per-edge h|d1 rows come from one
      dma_gather per edge (512 B rows, lo/hi split for int16 idx).
      s1[dst] is NOT gathered: dst rows of a tile are contiguous, so a
      [128,8] s-tile is DMA'd and broadcast per edge on the PE via
      S_T = transpose(S) followed by se = S_T.T @ s_tile (exact: S is
      one-hot).  Aggregation per chunk of 128 edges: S.T @ [feat*ex|ex]
      accumulated in PSUM; then normalize, ELU, W2 matmul -> table2slab.
  B:  layer-1 edge phase.  feat2|d2[src] via one pair-row dma_gather
      per edge (int16 idx = src>>1 + parity select).  s2[dst] via the
      same PE one-hot broadcast trick.  log_softmax tail.

Softmax max-subtraction is skipped: logits are O(0.3) so exp() is
stable, and softmax is shift-invariant.
"""

import os
import sys
from contextlib import ExitStack

import numpy as np
import ml_dtypes

for _p in ("/opt/trn_rl_repo",):
    if os.path.isdir(_p) and _p not in sys.path:
        sys.path.insert(0, _p)

import concourse.bass as bass
import concourse.bacc as bacc
import concourse.tile as tile
from concourse import mybir
from concourse import bass_utils
from concourse._compat import with_exitstack

F32 = mybir.dt.float32
BF16 = mybir.dt.bfloat16
I32 = mybir.dt.int32
I16 = mybir.dt.int16
AF = mybir.ActivationFunctionType
OP = mybir.AluOpType
P = 128
BF = ml_dtypes.bfloat16


class Cfg:
    def __init__(self, N, E, ncores, split=32768, neg=0.2, in_ch=128,
                 f=128, heads=8, hid=16, out=16):
        self.N = N
        self.E = E
        self.NCORES = ncores
        self.SPLIT = split
        self.NEG = neg
        self.IN = in_ch
        self.F = f
        self.H = heads
        self.HID = hid
        self.OUT = out
        assert N % ncores == 0
        self.NPC = N // ncores
        self.TPC = (self.NPC + P - 1) // P
        self.NPC_PAD = self.TPC * P
        self.NTILES = ncores * self.TPC
        self.N_PAD = self.NTILES * P
        self.NCHL_T = None
        self.NCHH_T = None
        self.NCH_T = None
        self.NCH = None


def _wrap16(vals):
    """[n] slot-ordered values -> [128, n//16] int16 wrapped layout."""
    n = vals.shape[0]
    assert n % 16 == 0
    w = vals.reshape(-1, 16).T.astype(np.int16)      # [16, n//16]
    return np.ascontiguousarray(np.tile(w, (8, 1)))  # [128, n//16]


def _prep_graph(cfg, edge_index):
    N, NPC, SPL = cfg.N, cfg.NPC, cfg.SPLIT
    src = np.concatenate([edge_index[0], np.arange(N, dtype=np.int64)])
    dst = np.concatenate([edge_index[1], np.arange(N, dtype=np.int64)])
    core = dst // NPC
    ld = dst - core * NPC
    tile_id = ld // P
    dstloc = ld % P
    hi = (src >= SPL).astype(np.int64)
    order = np.lexsort((src, hi, tile_id, core))
    src, dst, core, tile_id, dstloc, hi = (a[order] for a in
                                           (src, dst, core, tile_id,
                                            dstloc, hi))
    keyf = (core * cfg.TPC + tile_id) * 2 + hi
    cntf = np.bincount(keyf, minlength=cfg.NCORES * cfg.TPC * 2)
    cnt_lo = cntf[0::2].reshape(cfg.NCORES, cfg.TPC)
    cnt_hi = cntf[1::2].reshape(cfg.NCORES, cfg.TPC)
    cfg.NCHL_T = max(1, int(np.max((cnt_lo + P - 1) // P)))
    cfg.NCHH_T = max(1, int(np.max((cnt_hi + P - 1) // P)))
    cfg.NCH_T = cfg.NCHL_T + cfg.NCHH_T
    cfg.NCH = cfg.NCH_T * cfg.TPC
    starts = np.concatenate([[0], np.cumsum(cntf)])

    pc = dict(srcw_lo=[], srcw_hi=[], srcw=[], dstloc_f=[], src_par=[])
    for c in range(cfg.NCORES):
        ns_lo = cfg.TPC * cfg.NCHL_T * P
        ns_hi = cfg.TPC * cfg.NCHH_T * P
        ns = cfg.NCH * P
        v_srclo = np.zeros(ns_lo, np.int64)
        v_srchi = np.zeros(ns_hi, np.int64)
        v_src = np.zeros(ns, np.int64)          # src>>1 per slot
        v_dstloc = np.full(ns, -1.0, np.float32)
        v_spar = np.ones(ns, np.float32)
        for t in range(cfg.TPC):
            for h in (0, 1):
                k = ((c * cfg.TPC + t) * 2 + h)
                n = int(cntf[k])
                if n == 0:
                    continue
                sl = slice(starts[k], starts[k] + n)
                e_src = src[sl]
                e_dl = dstloc[sl]
                pos = np.arange(n)
                if h == 0:
                    v_srclo[t * cfg.NCHL_T * P + pos] = e_src
                    ch = t * cfg.NCH_T + pos // P
                else:
                    v_srchi[t * cfg.NCHH_T * P + pos] = e_src - SPL
                    ch = t * cfg.NCH_T + cfg.NCHL_T + pos // P
                slot = ch * P + pos % P
                v_src[slot] = e_src >> 1
                v_dstloc[slot] = e_dl
                v_spar[slot] = 1.0 - (e_src & 1)
        pc["srcw_lo"].append(_wrap16(v_srclo))
        pc["srcw_hi"].append(_wrap16(v_srchi))
        pc["srcw"].append(_wrap16(v_src))
        # slot arrays in [128, NCH] layout: slot = ch*128 + p -> [p, ch]
        pc["dstloc_f"].append(
            np.ascontiguousarray(v_dstloc.reshape(cfg.NCH, P).T))
        pc["src_par"].append(np.ascontiguousarray(
            v_spar.reshape(cfg.NCH, P).T))
    return pc


def _blockdiag_att(att, heads, hid, f):
    A = np.zeros((f, heads), dtype=np.float32)
    for h in range(heads):
        A[h * hid:(h + 1) * hid, h] = att[0, h]
    return A


def _ap(base, ap_list, off_extra=0):
    return bass.AP(tensor=base.tensor, offset=base.offset + off_extra,
                   ap=ap_list)


@with_exitstack
def _build_a1(ctx, tc, cfg, t):
    """Sharded node phase: this core's TPC tiles only."""
    nc = tc.nc
    WCOLS = cfg.F + 2 * cfg.H             # 144 matmul out cols
    TCOLS = cfg.F + cfg.H                 # 136 table1 used cols

    consts = ctx.enter_context(tc.tile_pool(name="consts", bufs=1))
    wpack = consts.tile([P, WCOLS], F32)
    nc.sync.dma_start(out=wpack[:], in_=t["wpack"][:, :])

    NT = cfg.TPC
    BLK = 8
    xpool = ctx.enter_context(tc.tile_pool(name="xt", bufs=2))
    npsum = ctx.enter_context(tc.tile_pool(name="npsum", bufs=2,
                                           space="PSUM"))
    nstage = ctx.enter_context(tc.tile_pool(name="nstage", bufs=3))
    for blk in range((NT + BLK - 1) // BLK):
        nt0 = blk * BLK
        nt1 = min(nt0 + BLK, NT)
        xt = xpool.tile([P, BLK * P], F32, tag="xt")
        nc.sync.dma_start(out=xt[:, 0:(nt1 - nt0) * P],
                          in_=t["xTs"][:, nt0 * P:nt1 * P])
        for j in range(nt1 - nt0):
            nt = nt0 + j
            pt = npsum.tile([P, WCOLS], F32, tag="npt")
            nc.tensor.matmul(out=pt[:], lhsT=xt[:, j * P:(j + 1) * P],
                             rhs=wpack[:], start=True, stop=True)
            s1 = nstage.tile([P, TCOLS], BF16, tag="s1")
            nc.scalar.activation(s1[:], pt[:, 0:TCOLS], AF.Copy)
            ss = nstage.tile([P, cfg.H], BF16, tag="ss")
            nc.scalar.activation(ss[:], pt[:, TCOLS:WCOLS], AF.Copy)
            nc.sync.dma_start(
                out=t["t1slab"][nt * P:(nt + 1) * P, 0:TCOLS], in_=s1[:])
            nc.sync.dma_start(
                out=t["sslab"][nt * P:(nt + 1) * P, 0:cfg.H], in_=ss[:])


@with_exitstack
def _build_a2(ctx, tc, cfg, t):
    nc = tc.nc
    NCH_T, NCHL_T, NCHH_T, TPC = cfg.NCH_T, cfg.NCHL_T, cfg.NCHH_T, cfg.TPC
    MCOLS = cfg.F + cfg.H                 # 136 message cols
    ROW1 = 256                            # table1 row elems (bf16)
    W2COLS = cfg.OUT + 2

    consts = ctx.enter_context(tc.tile_pool(name="consts", bufs=1))
    w2pack = consts.tile([P, W2COLS], F32)
    nc.sync.dma_start(out=w2pack[:], in_=t["w2pack"][:, :])
    iota = consts.tile([P, P], F32)
    nc.sync.dma_start(out=iota[:], in_=t["iota"][:, :])
    ident = consts.tile([P, P], F32)
    nc.sync.dma_start(out=ident[:], in_=t["ident"][:, :])
    identb = consts.tile([P, P], BF16)
    nc.sync.dma_start(out=identb[:], in_=t["identb"][:, :])

    gpool = ctx.enter_context(tc.tile_pool(name="g", bufs=3))
    ipool = ctx.enter_context(tc.tile_pool(name="idx", bufs=3))
    spool = ctx.enter_context(tc.tile_pool(name="sel", bufs=3))
    mpool = ctx.enter_context(tc.tile_pool(name="msg", bufs=2))
    lpool = ctx.enter_context(tc.tile_pool(name="logit", bufs=2))
    apsum = ctx.enter_context(tc.tile_pool(name="apsum", bufs=2,
                                           space="PSUM"))
    stpsum = ctx.enter_context(tc.tile_pool(name="stpsum", bufs=2,
                                            space="PSUM"))
    sepsum = ctx.enter_context(tc.tile_pool(name="sepsum", bufs=2,
                                            space="PSUM"))
    tpsum = ctx.enter_context(tc.tile_pool(name="tpsum", bufs=1,
                                           space="PSUM"))
    t2psum = ctx.enter_context(tc.tile_pool(name="t2psum", bufs=1,
                                            space="PSUM"))
    stsb = ctx.enter_context(tc.tile_pool(name="stsb", bufs=3))
    hpool = ctx.enter_context(tc.tile_pool(name="h1", bufs=2))

    tab_hi = t["table1"][cfg.SPLIT:cfg.N_PAD, :]

    for ti in range(TPC):
        c0 = ti * NCH_T
        il = ipool.tile([P, NCHL_T * 8], I16, tag="il")
        nc.sync.dma_start(out=il[:], in_=t["srcw_lo"][
            :, ti * NCHL_T * 8:(ti + 1) * NCHL_T * 8])
        ih = ipool.tile([P, NCHH_T * 8], I16, tag="ih")
        nc.sync.dma_start(out=ih[:], in_=t["srcw_hi"][
            :, ti * NCHH_T * 8:(ti + 1) * NCHH_T * 8])
        dloc = ipool.tile([P, NCH_T], F32, tag="dloc")
        nc.sync.dma_start(out=dloc[:], in_=t["dstloc_f"][:, c0:c0 + NCH_T])
        s_tile = ipool.tile([P, cfg.H], BF16, tag="stile")
        nc.sync.dma_start(out=s_tile[:],
                          in_=t["sslab"][ti * P:(ti + 1) * P, 0:cfg.H])

        # gathers (h|d1 by src)
        G = gpool.tile([P, NCH_T, ROW1], BF16, tag="G")
        nc.gpsimd.dma_gather(
            out_ap=G[:, 0:NCHL_T, :], in_ap=t["table1"][:, :],
            idxs_ap=il[:], num_idxs=NCHL_T * P, num_idxs_reg=NCHL_T * P,
            elem_size=ROW1, single_packet=False)
        nc.gpsimd.dma_gather(
            out_ap=G[:, NCHL_T:NCH_T, :], in_ap=tab_hi,
            idxs_ap=ih[:], num_idxs=NCHH_T * P, num_idxs_reg=NCHH_T * P,
            elem_size=ROW1, single_packet=False)

        # one-hot S
        S = spool.tile([P, NCH_T, P], BF16, tag="S")
        nc.vector.tensor_tensor(
            out=S[:],
            in0=_ap(iota[:], [iota[:].ap[0], [0, NCH_T], [1, P]]),
            in1=_ap(dloc[:], [dloc[:].ap[0], [1, NCH_T], [0, P]]),
            op=OP.is_equal)

        # s1[dst] per edge via PE: se = (S_k)^T.T @ s_tile
        sesb = spool.tile([P, NCH_T, cfg.H], F32, tag="sesb")
        for k in range(NCH_T):
            st_ps = stpsum.tile([P, P], BF16, tag="st")
            nc.tensor.transpose(out=st_ps[:], in_=S[:, k, :],
                                identity=identb[:])
            st_sb = stsb.tile([P, P], BF16, tag="stsb")
            nc.scalar.activation(st_sb[:], st_ps[:], AF.Copy)
            se_ps = sepsum.tile([P, cfg.H], F32, tag="se")
            nc.tensor.matmul(out=se_ps[:], lhsT=st_sb[:], rhs=s_tile[:],
                             start=True, stop=True)
            nc.scalar.activation(sesb[:, k, :], se_ps[:], AF.Copy)

        # logits -> ex (bf16)
        dcp = lpool.tile([P, NCH_T, cfg.H], F32, tag="dcp")
        nc.scalar.activation(dcp[:], G[:, :, cfg.F:cfg.F + cfg.H], AF.Copy)
        u = lpool.tile([P, NCH_T, cfg.H], F32, tag="u")
        nc.vector.tensor_tensor(out=u[:], in0=sesb[:], in1=dcp[:], op=OP.add)
        a = lpool.tile([P, NCH_T, cfg.H], F32, tag="a")
        nc.vector.scalar_tensor_tensor(out=a[:], in0=u[:], scalar=cfg.NEG,
                                       in1=u[:], op0=OP.mult, op1=OP.max)
        ex = lpool.tile([P, NCH_T, cfg.H], BF16, tag="ex")
        nc.scalar.activation(ex[:], a[:], AF.Exp)

        # Msg = [feat * ex | ex]
        M = mpool.tile([P, NCH_T, MCOLS], BF16, tag="M")
        nc.scalar.activation(M[:, :, cfg.F:MCOLS], ex[:], AF.Copy)
        nc.vector.tensor_tensor(
            out=_ap(M[:], [M[:].ap[0], [MCOLS, NCH_T], [cfg.HID, cfg.H],
                           [1, cfg.HID]]),
            in0=_ap(G[:], [G[:].ap[0], [ROW1, NCH_T], [cfg.HID, cfg.H],
                           [1, cfg.HID]]),
            in1=_ap(ex[:], [ex[:].ap[0], [cfg.H, NCH_T], [1, cfg.H],
                            [0, cfg.HID]]),
            op=OP.mult)

        # aggregate
        agg = apsum.tile([P, MCOLS], F32, tag="agg")
        for k in range(NCH_T):
            nc.tensor.matmul(out=agg[:], lhsT=S[:, k, :], rhs=M[:, k, :],
                             start=(k == 0), stop=(k == NCH_T - 1))

        # normalize + elu + feat2/d2/s2 slab
        den = hpool.tile([P, cfg.H], F32, tag="den")
        nc.vector.tensor_scalar_add(den[:], agg[:, cfg.F:MCOLS], 1e-20)
        rcp = hpool.tile([P, cfg.H], F32, tag="rcp")
        nc.vector.reciprocal(rcp[:], den[:])
        h1 = hpool.tile([P, cfg.F], F32, tag="h1")
        nc.vector.tensor_tensor(
            out=_ap(h1[:], [h1[:].ap[0], [cfg.HID, cfg.H], [1, cfg.HID]]),
            in0=_ap(agg[:], [agg[:].ap[0], [cfg.HID, cfg.H], [1, cfg.HID]]),
            in1=_ap(rcp[:], [rcp[:].ap[0], [1, cfg.H], [0, cfg.HID]]),
            op=OP.mult)
        neg = hpool.tile([P, cfg.F], F32, tag="neg")
        nc.vector.tensor_scalar_min(neg[:], h1[:], 0.0)
        pos = hpool.tile([P, cfg.F], F32, tag="pos")
        nc.vector.tensor_scalar_max(pos[:], h1[:], 0.0)
        een = hpool.tile([P, cfg.F], F32, tag="een")
        nc.scalar.activation(een[:], neg[:], AF.Exp)
        elu = hpool.tile([P, cfg.F], F32, tag="elu")
        nc.vector.scalar_tensor_tensor(out=elu[:], in0=een[:], scalar=-1.0,
                                       in1=pos[:], op0=OP.add, op1=OP.add)
        eT_ps = tpsum.tile([P, P], F32, tag="eT")
        nc.tensor.transpose(out=eT_ps[:], in_=elu[:], identity=ident[:])
        eT = hpool.tile([P, P], F32, tag="eTs")
        nc.scalar.activation(eT[:], eT_ps[:], AF.Copy)
        t2 = t2psum.tile([P, W2COLS], F32, tag="t2")
        nc.tensor.matmul(out=t2[:], lhsT=eT[:], rhs=w2pack[:],
                         start=True, stop=True)
        t2s = hpool.tile([P, W2COLS], F32, tag="t2s")
        nc.scalar.activation(t2s[:], t2[:], AF.Copy)
        nc.sync.dma_start(out=t["table2slab"][ti * P:(ti + 1) * P, :],
                          in_=t2s[:])


@with_exitstack
def _build_b(ctx, tc, cfg, t):
    nc = tc.nc
    NCH_T, TPC = cfg.NCH_T, cfg.TPC
    UC = cfg.OUT + 2                    # 18 used cols in table2
    MC = cfg.OUT + 1                    # 17 message cols

    consts = ctx.enter_context(tc.tile_pool(name="consts", bufs=1))
    iota = consts.tile([P, P], F32)
    nc.sync.dma_start(out=iota[:], in_=t["iota"][:, :])
    identb = consts.tile([P, P], BF16)
    nc.sync.dma_start(out=identb[:], in_=t["identb"][:, :])

    ipool = ctx.enter_context(tc.tile_pool(name="idx", bufs=3))
    gpool = ctx.enter_context(tc.tile_pool(name="g2", bufs=3))
    spool = ctx.enter_context(tc.tile_pool(name="s2", bufs=3))
    lpool = ctx.enter_context(tc.tile_pool(name="l2", bufs=2))
    mpool = ctx.enter_context(tc.tile_pool(name="m2", bufs=2))
    apsum = ctx.enter_context(tc.tile_pool(name="aps2", bufs=2,
                                           space="PSUM"))
    stps = ctx.enter_context(tc.tile_pool(name="stps2", bufs=2,
                                          space="PSUM"))
    seps = ctx.enter_context(tc.tile_pool(name="seps2", bufs=2,
                                          space="PSUM"))
    stsb = ctx.enter_context(tc.tile_pool(name="stsb2", bufs=3))
    opool = ctx.enter_context(tc.tile_pool(name="o", bufs=3))

    tab_pair = _ap(t["table2"][:, :], [[256, cfg.N_PAD // 2], [1, 256]])

    for ti in range(TPC):
        c0 = ti * NCH_T
        isrc = ipool.tile([P, NCH_T * 8], I16, tag="isrc")
        nc.sync.dma_start(out=isrc[:], in_=t["srcw"][
            :, ti * NCH_T * 8:(ti + 1) * NCH_T * 8])
        dloc = ipool.tile([P, NCH_T], F32, tag="dloc")
        nc.sync.dma_start(out=dloc[:], in_=t["dstloc_f"][:, c0:c0 + NCH_T])
        spar = ipool.tile([P, NCH_T], F32, tag="spar")
        nc.sync.dma_start(out=spar[:], in_=t["src_par"][:, c0:c0 + NCH_T])
        s2_tile = ipool.tile([P, 8], BF16, tag="s2tile")
        nc.sync.dma_start(out=s2_tile[:],
                          in_=t["s2slab"][ti * P:(ti + 1) * P, 0:8])

        G2 = gpool.tile([P, NCH_T, 256], BF16, tag="G2")
        nc.gpsimd.dma_gather(
            out_ap=G2[:], in_ap=tab_pair, idxs_ap=isrc[:],
            num_idxs=NCH_T * P, num_idxs_reg=NCH_T * P, elem_size=256,
            single_packet=False)

        # parity selects: x = odd + par*(even - odd)
        Rd = spool.tile([P, NCH_T, UC], F32, tag="Rd")
        nc.vector.tensor_tensor(out=Rd[:], in0=G2[:, :, 0:UC],
                                in1=G2[:, :, 128:128 + UC], op=OP.subtract)
        Rm = spool.tile([P, NCH_T, UC], F32, tag="Rm")
        nc.vector.tensor_tensor(
            out=Rm[:], in0=Rd[:],
            in1=_ap(spar[:], [spar[:].ap[0], [1, NCH_T], [0, UC]]),
            op=OP.mult)
        Ro = spool.tile([P, NCH_T, UC], F32, tag="Ro")
        nc.scalar.activation(Ro[:], G2[:, :, 128:128 + UC], AF.Copy)
        R = spool.tile([P, NCH_T, UC], F32, tag="R")
        nc.vector.tensor_tensor(out=R[:], in0=Rm[:], in1=Ro[:], op=OP.add)

        S = spool.tile([P, NCH_T, P], BF16, tag="S")
        nc.vector.tensor_tensor(
            out=S[:],
            in0=_ap(iota[:], [iota[:].ap[0], [0, NCH_T], [1, P]]),
            in1=_ap(dloc[:], [dloc[:].ap[0], [1, NCH_T], [0, P]]),
            op=OP.is_equal)

        # s2[dst] per edge via PE one-hot broadcast
        sesb = spool.tile([P, NCH_T, 1], F32, tag="sesb2")
        for k in range(NCH_T):
            st_ps = stps.tile([P, P], BF16, tag="st2")
            nc.tensor.transpose(out=st_ps[:], in_=S[:, k, :],
                                identity=identb[:])
            st_sb = stsb.tile([P, P], BF16, tag="stsb2")
            nc.scalar.activation(st_sb[:], st_ps[:], AF.Copy)
            se_ps = seps.tile([P, 1], F32, tag="se2")
            nc.tensor.matmul(out=se_ps[:], lhsT=st_sb[:],
                             rhs=s2_tile[:, 0:1], start=True, stop=True)
            nc.scalar.activation(sesb[:, k, :], se_ps[:], AF.Copy)

        u = lpool.tile([P, NCH_T, 1], F32, tag="u2")
        nc.vector.tensor_tensor(out=u[:], in0=sesb[:],
                                in1=R[:, :, cfg.OUT:MC], op=OP.add)
        a = lpool.tile([P, NCH_T, 1], F32, tag="a2")
        nc.vector.scalar_tensor_tensor(out=a[:], in0=u[:], scalar=cfg.NEG,
                                       in1=u[:], op0=OP.mult, op1=OP.max)
        ex = lpool.tile([P, NCH_T, 1], F32, tag="ex2")
        nc.scalar.activation(ex[:], a[:], AF.Exp)

        M = mpool.tile([P, NCH_T, MC], BF16, tag="M2")
        nc.scalar.activation(M[:, :, cfg.OUT:MC], ex[:], AF.Copy)
        nc.vector.tensor_tensor(
            out=M[:, :, 0:cfg.OUT],
            in0=R[:, :, 0:cfg.OUT],
            in1=_ap(ex[:], [ex[:].ap[0], [1, NCH_T], [0, cfg.OUT]]),
            op=OP.mult)

        agg = apsum.tile([P, MC], F32, tag="agg2")
        for k in range(NCH_T):
            nc.tensor.matmul(out=agg[:], lhsT=S[:, k, :], rhs=M[:, k, :],
                             start=(k == 0), stop=(k == NCH_T - 1))

        den = opool.tile([P, 1], F32, tag="den")
        nc.vector.tensor_scalar_add(den[:], agg[:, cfg.OUT:MC], 1e-20)
        rcp = opool.tile([P, 1], F32, tag="rcp")
        nc.vector.reciprocal(rcp[:], den[:])
        res = opool.tile([P, cfg.OUT + 1], F32, tag="res")
        nc.vector.tensor_tensor(
            out=res[:, 0:cfg.OUT], in0=agg[:, 0:cfg.OUT],
            in1=_ap(rcp[:], [rcp[:].ap[0], [0, cfg.OUT]]), op=OP.mult)
        # logits are O(0.3): exp() safe without max-subtraction; host
        # finishes log_softmax as h2 - log(sum_exp).
        pe = opool.tile([P, cfg.OUT], F32, tag="pe")
        nc.scalar.activation(pe[:], res[:, 0:cfg.OUT], AF.Exp,
                             accum_out=res[:, cfg.OUT:cfg.OUT + 1])
        nc.sync.dma_start(out=t["outp"][ti * P:(ti + 1) * P, :], in_=res[:])


def _decl_a1(nc, cfg):
    t = {}
    WCOLS = cfg.F + 2 * cfg.H

    def inp(name, shape, dt):
        t[name] = nc.dram_tensor(name, shape, dt, kind="ExternalInput").ap()

    inp("xTs", [P, cfg.NPC_PAD], F32)
    inp("wpack", [P, WCOLS], F32)
    t["t1slab"] = nc.dram_tensor("t1slab", [cfg.NPC_PAD, 256], BF16,
                                 kind="ExternalOutput").ap()
    t["sslab"] = nc.dram_tensor("sslab", [cfg.NPC_PAD, 8], BF16,
                                kind="ExternalOutput").ap()
    return t


def _decl_a2(nc, cfg):
    t = {}
    W2COLS = cfg.OUT + 2

    def inp(name, shape, dt):
        t[name] = nc.dram_tensor(name, shape, dt, kind="ExternalInput").ap()

    inp("table1", [cfg.N_PAD, 256], BF16)
    inp("sslab", [cfg.NPC_PAD, 8], BF16)
    inp("w2pack", [P, W2COLS], F32)
    inp("iota", [P, P], F32)
    inp("ident", [P, P], F32)
    inp("identb", [P, P], BF16)
    inp("srcw_lo", [P, cfg.TPC * cfg.NCHL_T * 8], I16)
    inp("srcw_hi", [P, cfg.TPC * cfg.NCHH_T * 8], I16)
    inp("dstloc_f", [P, cfg.NCH], F32)
    t["table2slab"] = nc.dram_tensor("table2slab", [cfg.NPC_PAD, W2COLS],
                                     F32, kind="ExternalOutput").ap()
    return t


def _decl_b(nc, cfg):
    t = {}

    def inp(name, shape, dt):
        t[name] = nc.dram_tensor(name, shape, dt, kind="ExternalInput").ap()

    inp("table2", [cfg.N_PAD, 128], BF16)
    inp("s2slab", [cfg.NPC_PAD, 8], BF16)
    inp("srcw", [P, cfg.NCH * 8], I16)
    inp("dstloc_f", [P, cfg.NCH], F32)
    inp("src_par", [P, cfg.NCH], F32)
    inp("iota", [P, P], F32)
    inp("identb", [P, P], BF16)
    t["outp"] = nc.dram_tensor("outp", [cfg.NPC_PAD, cfg.OUT + 1], F32,
                               kind="ExternalOutput").ap()
    return t


def _compile(build_fn, decl_fn, cfg):
    nc = bacc.Bacc("TRN2", target_bir_lowering=False, debug=False,
                   enable_asserts=False, num_devices=cfg.NCORES)
    t = decl_fn(nc, cfg)
    with tile.TileContext(nc) as tc:
        build_fn(tc, cfg, t)
    nc.compile()
    return nc


def _host_prep_weights(cfg, W1, att_src1, att_dst1, W2, att_src2, att_dst2):
    A_d1 = _blockdiag_att(np.asarray(att_dst1, np.float32), cfg.H, cfg.HID,
                          cfg.F)
    A_s1 = _blockdiag_att(np.asarray(att_src1, np.float32), cfg.H, cfg.HID,
                          cfg.F)
    W1T = np.asarray(W1, np.float32).T.copy()
    wpack = np.concatenate([W1T, W1T @ A_d1, W1T @ A_s1], axis=1)
    W2T = np.asarray(W2, np.float32).T.copy()
    a_d2 = np.asarray(att_dst2, np.float32).reshape(cfg.OUT, 1)
    a_s2 = np.asarray(att_src2, np.float32).reshape(cfg.OUT, 1)
    w2pack = np.concatenate([W2T, W2T @ a_d2, W2T @ a_s2], axis=1)
    return (np.ascontiguousarray(wpack, np.float32),
            np.ascontiguousarray(w2pack, np.float32))


_CACHE = {}


def _get_kernels(cfg):
    key = (cfg.N, cfg.E, cfg.NCORES, cfg.NCH_T, cfg.NCHL_T)
    if key not in _CACHE:
        nca1 = _compile(_build_a1, _decl_a1, cfg)
        nca2 = _compile(_build_a2, _decl_a2, cfg)
        ncb = _compile(_build_b, _decl_b, cfg)
        _CACHE[key] = (nca1, nca2, ncb)
    return _CACHE[key]


def run(cfg, inputs, runner=None):
    x = np.asarray(inputs["x"], np.float32)
    edge_index = np.asarray(inputs["edge_index"], np.int64)
    pc = _prep_graph(cfg, edge_index)
    wpack, w2pack = _host_prep_weights(
        cfg, inputs["W1"], inputs["att_src1"], inputs["att_dst1"],
        inputs["W2"], inputs["att_src2"], inputs["att_dst2"])

    xT = np.zeros((P, cfg.N_PAD), np.float32)
    xT[:, :cfg.N] = x.T
    iota = np.tile(np.arange(P, dtype=np.float32), (P, 1))
    ident = np.eye(P, dtype=np.float32)
    identb = np.eye(P, dtype=BF)

    nca1, nca2, ncb = _get_kernels(cfg)

    if runner is None:
        def runner(nc, in_maps):
            r = bass_utils.run_bass_kernel_spmd(
                nc, in_maps, core_ids=list(range(cfg.NCORES)))
            return r.results

    # --- A1: sharded node phase ---
    in_maps_a1 = []
    for c in range(cfg.NCORES):
        xTs = np.zeros((P, cfg.NPC_PAD), np.float32)
        n0 = c * cfg.NPC
        n1 = min(n0 + cfg.NPC_PAD, cfg.N_PAD)
        xTs[:, 0:n1 - n0] = xT[:, n0:n1]
        in_maps_a1.append(dict(xTs=xTs, wpack=wpack))
    res_a1 = runner(nca1, in_maps_a1)

    table1 = np.zeros((cfg.N_PAD, 256), BF)
    sslabs = []
    for c in range(cfg.NCORES):
        t1s = np.asarray(res_a1[c]["t1slab"])
        n0 = c * cfg.NPC
        n1 = min(n0 + cfg.NPC_PAD, cfg.N_PAD)
        take = min(cfg.NPC, n1 - n0)
        table1[n0:n0 + take] = t1s[:take]
        sslabs.append(np.asarray(res_a1[c]["sslab"]))

    # --- A2: layer-0 edge phase ---
    in_maps_a2 = []
    for c in range(cfg.NCORES):
        in_maps_a2.append(dict(
            table1=table1, sslab=sslabs[c], w2pack=w2pack, iota=iota,
            ident=ident, identb=identb,
            srcw_lo=pc["srcw_lo"][c], srcw_hi=pc["srcw_hi"][c],
            dstloc_f=pc["dstloc_f"][c]))
    res_a2 = runner(nca2, in_maps_a2)

    table2 = np.zeros((cfg.N_PAD, 128), BF)
    s2slabs = []
    for c in range(cfg.NCORES):
        slab = np.asarray(res_a2[c]["table2slab"], np.float32)
        table2[c * cfg.NPC:(c + 1) * cfg.NPC, 0:cfg.OUT + 2] = \
            slab[:cfg.NPC].astype(BF)
        s2s = np.zeros((cfg.NPC_PAD, 8), BF)
        s2s[:, 0] = slab[:, cfg.OUT + 1].astype(BF)
        s2slabs.append(s2s)

    # --- B: layer-1 edge phase ---
    in_maps_b = []
    for c in range(cfg.NCORES):
        in_maps_b.append(dict(
            table2=table2, s2slab=s2slabs[c], srcw=pc["srcw"][c],
            dstloc_f=pc["dstloc_f"][c], src_par=pc["src_par"][c],
            iota=iota, identb=identb))
    res_b = runner(ncb, in_maps_b)

    out = np.zeros((cfg.N, cfg.OUT), np.float32)
    for c in range(cfg.NCORES):
        ob = np.asarray(res_b[c]["outp"], np.float32)[:cfg.NPC]
        out[c * cfg.NPC:(c + 1) * cfg.NPC] = \
            ob[:, 0:cfg.OUT] - np.log(ob[:, cfg.OUT:cfg.OUT + 1])
    return out


def kernel(**inputs):
    cfg = Cfg(N=50000, E=1600000, ncores=8)
    return run(cfg, inputs)


# revision 9
# speedup vs baseline: 1.3641x; 1.2405x over previous
"""
2-layer GAT on Trainium2 (8 NeuronCores, SPMD via bass/Tile).

Sharding: destination nodes are block-sharded across the 8 cores (6250
nodes each).  All per-edge work runs on the core owning the edge's dst.

Pipeline = three bass kernels with host gathers between them:
  A1: node phase, sharded — core c computes h = x@W1pack for its own
      ~6272 nodes only, writes t1slab [h|d1] (bf16) + sslab s1 (bf16).
      Host assembles the full table1 from the 8 slabs.
  A2: layer-0 edge phase.  Per-edge h|d1 rows come from one
      dma_gather per edge (512 B rows, lo/hi split for int16 idx).
      s1[dst] is NOT gathered: dst rows of a tile are contiguous, so a
      [128,8] s-tile is DMA'd and broadcast per edge on the PE via
      S_T = transpose(S) followed by se = S_T.T @ s_tile (exact: S is
      one-hot).  Aggregation per chunk of 128 edges: S.T @ [feat*ex|ex]
      accumulated in PSUM; then normalize, ELU, W2 matmul -> table2slab.
  B:  layer-1 edge phase.  feat2|d2[src] via one pair-row dma_gather
      per edge (int16 idx = src>>1 + parity select).  s2[dst] via the
      same PE one-hot broadcast trick.  log_softmax tail.

Softmax max-subtraction is skipped: logits are O(0.3) so exp() is
stable, and softmax is shift-invariant.
"""

import os
import sys
from contextlib import ExitStack

import numpy as np
import ml_dtypes

for _p in ("/opt/trn_rl_repo",):
    if os.path.isdir(_p) and _p not in sys.path:
        sys.path.insert(0, _p)

import concourse.bass as bass
import concourse.bacc as bacc
import concourse.tile as tile
from concourse import mybir
from concourse import bass_utils
from concourse._compat import with_exitstack

F32 = mybir.dt.float32
BF16 = mybir.dt.bfloat16
I32 = mybir.dt.int32
I16 = mybir.dt.int16
AF = mybir.ActivationFunctionType
OP = mybir.AluOpType
P = 128
BF = ml_dtypes.bfloat16


class Cfg:
    def __init__(self, N, E, ncores, split=32768, neg=0.2, in_ch=128,
                 f=128, heads=8, hid=16, out=16):
        self.N = N
        self.E = E
        self.NCORES = ncores
        self.SPLIT = split
        self.NEG = neg
        self.IN = in_ch
        self.F = f
        self.H = heads
        self.HID = hid
        self.OUT = out
        assert N % ncores == 0
        self.NPC = N // ncores
        self.TPC = (self.NPC + P - 1) // P
        self.NPC_PAD = self.TPC * P
        self.NTILES = ncores * self.TPC
        self.N_PAD = self.NTILES * P
        self.NCHL_T = None
        self.NCHH_T = None
        self.NCH_T = None
        self.NCH = None


def _wrap16(vals):
    """[n] slot-ordered values -> [128, n//16] int16 wrapped layout."""
    n = vals.shape[0]
    assert n % 16 == 0
    w = vals.reshape(-1, 16).T.astype(np.int16)      # [16, n//16]
    return np.ascontiguousarray(np.tile(w, (8, 1)))  # [128, n//16]


def _prep_graph(cfg, edge_index):
    N, NPC, SPL = cfg.N, cfg.NPC, cfg.SPLIT
    src = np.concatenate([edge_index[0], np.arange(N, dtype=np.int64)])
    dst = np.concatenate([edge_index[1], np.arange(N, dtype=np.int64)])
    core = dst // NPC
    ld = dst - core * NPC
    tile_id = ld // P
    dstloc = ld % P
    hi = (src >= SPL).astype(np.int64)
    order = np.lexsort((src, hi, tile_id, core))
    src, dst, core, tile_id, dstloc, hi = (a[order] for a in
                                           (src, dst, core, tile_id,
                                            dstloc, hi))
    keyf = (core * cfg.TPC + tile_id) * 2 + hi
    cntf = np.bincount(keyf, minlength=cfg.NCORES * cfg.TPC * 2)
    cnt_lo = cntf[0::2].reshape(cfg.NCORES, cfg.TPC)
    cnt_hi = cntf[1::2].reshape(cfg.NCORES, cfg.TPC)
    cfg.NCHL_T = max(1, int(np.max((cnt_lo + P - 1) // P)))
    cfg.NCHH_T = max(1, int(np.max((cnt_hi + P - 1) // P)))
    cfg.NCH_T = cfg.NCHL_T + cfg.NCHH_T
    cfg.NCH = cfg.NCH_T * cfg.TPC
    starts = np.concatenate([[0], np.cumsum(cntf)])

    pc = dict(srcw_lo=[], srcw_hi=[], srcw=[], dstloc_f=[], src_par=[])
    for c in range(cfg.NCORES):
        ns_lo = cfg.TPC * cfg.NCHL_T * P
        ns_hi = cfg.TPC * cfg.NCHH_T * P
        ns = cfg.NCH * P
        v_srclo = np.zeros(ns_lo, np.int64)
        v_srchi = np.zeros(ns_hi, np.int64)
        v_src = np.zeros(ns, np.int64)          # src>>1 per slot
        v_dstloc = np.full(ns, -1.0, np.float32)
        v_spar = np.ones(ns, np.float32)
        for t in range(cfg.TPC):
            for h in (0, 1):
                k = ((c * cfg.TPC + t) * 2 + h)
                n = int(cntf[k])
                if n == 0:
                    continue
                sl = slice(starts[k], starts[k] + n)
                e_src = src[sl]
                e_dl = dstloc[sl]
                pos = np.arange(n)
                if h == 0:
                    v_srclo[t * cfg.NCHL_T * P + pos] = e_src
                    ch = t * cfg.NCH_T + pos // P
                else:
                    v_srchi[t * cfg.NCHH_T * P + pos] = e_src - SPL
                    ch = t * cfg.NCH_T + cfg.NCHL_T + pos // P
                slot = ch * P + pos % P
                v_src[slot] = e_src >> 1
                v_dstloc[slot] = e_dl
                v_spar[slot] = 1.0 - (e_src & 1)
        pc["srcw_lo"].append(_wrap16(v_srclo))
        pc["srcw_hi"].append(_wrap16(v_srchi))
        pc["srcw"].append(_wrap16(v_src))
        # slot arrays in [128, NCH] layout: slot = ch*128 + p -> [p, ch]
        pc["dstloc_f"].append(
            np.ascontiguousarray(v_dstloc.reshape(cfg.NCH, P).T))
        pc["src_par"].append(np.ascontiguousarray(
            v_spar.reshape(cfg.NCH, P).T))
    return pc


def _blockdiag_att(att, heads, hid, f):
    A = np.zeros((f, heads), dtype=np.float32)
    for h in range(heads):
        A[h * hid:(h + 1) * hid, h] = att[0, h]
    return A


def _ap(base, ap_list, off_extra=0):
    return bass.AP(tensor=base.tensor, offset=base.offset + off_extra,
                   ap=ap_list)


@with_exitstack
def _build_a1(ctx, tc, cfg, t):
    """Sharded node phase: this core's TPC tiles only."""
    nc = tc.nc
    WCOLS = cfg.F + 2 * cfg.H             # 144 matmul out cols
    TCOLS = cfg.F + cfg.H                 # 136 table1 used cols

    consts = ctx.enter_context(tc.tile_pool(name="consts", bufs=1))
    wpack = consts.tile([P, WCOLS], F32)
    nc.sync.dma_start(out=wpack[:], in_=t["wpack"][:, :])

    NT = cfg.TPC
    BLK = 8
    xpool = ctx.enter_context(tc.tile_pool(name="xt", bufs=2))
    npsum = ctx.enter_context(tc.tile_pool(name="npsum", bufs=2,
                                           space="PSUM"))
    nstage = ctx.enter_context(tc.tile_pool(name="nstage", bufs=3))
    for blk in range((NT + BLK - 1) // BLK):
        nt0 = blk * BLK
        nt1 = min(nt0 + BLK, NT)
        xt = xpool.tile([P, BLK * P], F32, tag="xt")
        nc.sync.dma_start(out=xt[:, 0:(nt1 - nt0) * P],
                          in_=t["xTs"][:, nt0 * P:nt1 * P])
        for j in range(nt1 - nt0):
            nt = nt0 + j
            pt = npsum.tile([P, WCOLS], F32, tag="npt")
            nc.tensor.matmul(out=pt[:], lhsT=xt[:, j * P:(j + 1) * P],
                             rhs=wpack[:], start=True, stop=True)
            s1 = nstage.tile([P, TCOLS], BF16, tag="s1")
            nc.scalar.activation(s1[:], pt[:, 0:TCOLS], AF.Copy)
            ss = nstage.tile([P, cfg.H], BF16, tag="ss")
            nc.scalar.activation(ss[:], pt[:, TCOLS:WCOLS], AF.Copy)
            nc.sync.dma_start(
                out=t["t1slab"][nt * P:(nt + 1) * P, 0:TCOLS], in_=s1[:])
            nc.sync.dma_start(
                out=t["sslab"][nt * P:(nt + 1) * P, 0:cfg.H], in_=ss[:])


@with_exitstack
def _build_a2(ctx, tc, cfg, t):
    nc = tc.nc
    NCH_T, NCHL_T, NCHH_T, TPC = cfg.NCH_T, cfg.NCHL_T, cfg.NCHH_T, cfg.TPC
    MCOLS = cfg.F + cfg.H                 # 136 message cols
    ROW1 = 256                            # table1 row elems (bf16)
    W2COLS = cfg.OUT + 2

    consts = ctx.enter_context(tc.tile_pool(name="consts", bufs=1))
    w2pack = consts.tile([P, W2COLS], F32)
    nc.sync.dma_start(out=w2pack[:], in_=t["w2pack"][:, :])
    iota = consts.tile([P, P], F32)
    nc.sync.dma_start(out=iota[:], in_=t["iota"][:, :])
    ident = consts.tile([P, P], F32)
    nc.sync.dma_start(out=ident[:], in_=t["ident"][:, :])
    identb = consts.tile([P, P], BF16)
    nc.sync.dma_start(out=identb[:], in_=t["identb"][:, :])

    gpool = ctx.enter_context(tc.tile_pool(name="g", bufs=3))
    ipool = ctx.enter_context(tc.tile_pool(name="idx", bufs=3))
    spool = ctx.enter_context(tc.tile_pool(name="sel", bufs=3))
    mpool = ctx.enter_context(tc.tile_pool(name="msg", bufs=2))
    lpool = ctx.enter_context(tc.tile_pool(name="logit", bufs=2))
    apsum = ctx.enter_context(tc.tile_pool(name="apsum", bufs=2,
                                           space="PSUM"))
    stpsum = ctx.enter_context(tc.tile_pool(name="stpsum", bufs=2,
                                            space="PSUM"))
    sepsum = ctx.enter_context(tc.tile_pool(name="sepsum", bufs=2,
                                            space="PSUM"))
    tpsum = ctx.enter_context(tc.tile_pool(name="tpsum", bufs=1,
                                           space="PSUM"))
    t2psum = ctx.enter_context(tc.tile_pool(name="t2psum", bufs=1,
                                            space="PSUM"))
    stsb = ctx.enter_context(tc.tile_pool(name="stsb", bufs=3))
    hpool = ctx.enter_context(tc.tile_pool(name="h1", bufs=2))

    tab_hi = t["table1"][cfg.SPLIT:cfg.N_PAD, :]

    for ti in range(TPC):
        c0 = ti * NCH_T
        il = ipool.tile([P, NCHL_T * 8], I16, tag="il")
        nc.sync.dma_start(out=il[:], in_=t["srcw_lo"][
            :, ti * NCHL_T * 8:(ti + 1) * NCHL_T * 8])
        ih = ipool.tile([P, NCHH_T * 8], I16, tag="ih")
        nc.sync.dma_start(out=ih[:], in_=t["srcw_hi"][
            :, ti * NCHH_T * 8:(ti + 1) * NCHH_T * 8])
        dloc = ipool.tile([P, NCH_T], F32, tag="dloc")
        nc.sync.dma_start(out=dloc[:], in_=t["dstloc_f"][:, c0:c0 + NCH_T])
        s_tile = ipool.tile([P, cfg.H], BF16, tag="stile")
        nc.sync.dma_start(out=s_tile[:],
                          in_=t["sslab"][ti * P:(ti + 1) * P, 0:cfg.H])

        # gathers (h|d1 by src)
        G = gpool.tile([P, NCH_T, ROW1], BF16, tag="G")
        nc.gpsimd.dma_gather(
            out_ap=G[:, 0:NCHL_T, :], in_ap=t["table1"][:, :],
            idxs_ap=il[:], num_idxs=NCHL_T * P, num_idxs_reg=NCHL_T * P,
            elem_size=ROW1, single_packet=False)
        nc.gpsimd.dma_gather(
            out_ap=G[:, NCHL_T:NCH_T, :], in_ap=tab_hi,
            idxs_ap=ih[:], num_idxs=NCHH_T * P, num_idxs_reg=NCHH_T * P,
            elem_size=ROW1, single_packet=False)

        # one-hot S
        S = spool.tile([P, NCH_T, P], BF16, tag="S")
        nc.vector.tensor_tensor(
            out=S[:],
            in0=_ap(iota[:], [iota[:].ap[0], [0, NCH_T], [1, P]]),
            in1=_ap(dloc[:], [dloc[:].ap[0], [1, NCH_T], [0, P]]),
            op=OP.is_equal)

        # s1[dst] per edge via PE: se = (S_k)^T.T @ s_tile
        sesb = spool.tile([P, NCH_T, cfg.H], F32, tag="sesb")
        for k in range(NCH_T):
            st_ps = stpsum.tile([P, P], BF16, tag="st")
            nc.tensor.transpose(out=st_ps[:], in_=S[:, k, :],
                                identity=identb[:])
            st_sb = stsb.tile([P, P], BF16, tag="stsb")
            nc.scalar.activation(st_sb[:], st_ps[:], AF.Copy)
            se_ps = sepsum.tile([P, cfg.H], F32, tag="se")
            nc.tensor.matmul(out=se_ps[:], lhsT=st_sb[:], rhs=s_tile[:],
                             start=True, stop=True)
            nc.scalar.activation(sesb[:, k, :], se_ps[:], AF.Copy)

        # logits -> ex (bf16)
        dcp = lpool.tile([P, NCH_T, cfg.H], F32, tag="dcp")
        nc.scalar.activation(dcp[:], G[:, :, cfg.F:cfg.F + cfg.H], AF.Copy)
        u = lpool.tile([P, NCH_T, cfg.H], F32, tag="u")
        nc.vector.tensor_tensor(out=u[:], in0=sesb[:], in1=dcp[:], op=OP.add)
        a = lpool.tile([P, NCH_T, cfg.H], F32, tag="a")
        nc.vector.scalar_tensor_tensor(out=a[:], in0=u[:], scalar=cfg.NEG,
                                       in1=u[:], op0=OP.mult, op1=OP.max)
        ex = lpool.tile([P, NCH_T, cfg.H], BF16, tag="ex")
        nc.scalar.activation(ex[:], a[:], AF.Exp)

        # Msg = [feat * ex | ex]
        M = mpool.tile([P, NCH_T, MCOLS], BF16, tag="M")
        nc.scalar.activation(M[:, :, cfg.F:MCOLS], ex[:], AF.Copy)
        nc.vector.tensor_tensor(
            out=_ap(M[:], [M[:].ap[0], [MCOLS, NCH_T], [cfg.HID, cfg.H],
                           [1, cfg.HID]]),
            in0=_ap(G[:], [G[:].ap[0], [ROW1, NCH_T], [cfg.HID, cfg.H],
                           [1, cfg.HID]]),
            in1=_ap(ex[:], [ex[:].ap[0], [cfg.H, NCH_T], [1, cfg.H],
                            [0, cfg.HID]]),
            op=OP.mult)

        # aggregate
        agg = apsum.tile([P, MCOLS], F32, tag="agg")
        for k in range(NCH_T):
            nc.tensor.matmul(out=agg[:], lhsT=S[:, k, :], rhs=M[:, k, :],
                             start=(k == 0), stop=(k == NCH_T - 1))

        # normalize + elu + feat2/d2/s2 slab
        den = hpool.tile([P, cfg.H], F32, tag="den")
        nc.vector.tensor_scalar_add(den[:], agg[:, cfg.F:MCOLS], 1e-20)
        rcp = hpool.tile([P, cfg.H], F32, tag="rcp")
        nc.vector.reciprocal(rcp[:], den[:])
        h1 = hpool.tile([P, cfg.F], F32, tag="h1")
        nc.vector.tensor_tensor(
            out=_ap(h1[:], [h1[:].ap[0], [cfg.HID, cfg.H], [1, cfg.HID]]),
            in0=_ap(agg[:], [agg[:].ap[0], [cfg.HID, cfg.H], [1, cfg.HID]]),
            in1=_ap(rcp[:], [rcp[:].ap[0], [1, cfg.H], [0, cfg.HID]]),
            op=OP.mult)
        neg = hpool.tile([P, cfg.F], F32, tag="neg")
        nc.vector.tensor_scalar_min(neg[:], h1[:], 0.0)
        pos = hpool.tile([P, cfg.F], F32, tag="pos")
        nc.vector.tensor_scalar_max(pos[:], h1[:], 0.0)
        een = hpool.tile([P, cfg.F], F32, tag="een")
        nc.scalar.activation(een[:], neg[:], AF.Exp)
        elu = hpool.tile([P, cfg.F], F32, tag="elu")
        nc.vector.scalar_tensor_tensor(out=elu[:], in0=een[:], scalar=-1.0,
                                       in1=pos[:], op0=OP.add, op1=OP.add)
        eT_ps = tpsum.tile([P, P], F32, tag="eT")
        nc.tensor.transpose(out=eT_ps[:], in_=elu[:], identity=ident[:])
        eT = hpool.tile([P, P], F32, tag="eTs")
        nc.scalar.activation(eT[:], eT_ps[:], AF.Copy)
        t2 = t2psum.tile([P, W2COLS], F32, tag="t2")
        nc.tensor.matmul(out=t2[:], lhsT=eT[:], rhs=w2pack[:],
                         start=True, stop=True)
        t2s = hpool.tile([P, W2COLS], F32, tag="t2s")
        nc.scalar.activation(t2s[:], t2[:], AF.Copy)
        nc.sync.dma_start(out=t["table2slab"][ti * P:(ti + 1) * P, :],
                          in_=t2s[:])


@with_exitstack
def _build_b(ctx, tc, cfg, t):
    nc = tc.nc
    NCH_T, TPC = cfg.NCH_T, cfg.TPC
    UC = cfg.OUT + 2                    # 18 used cols in table2
    MC = cfg.OUT + 1                    # 17 message cols

    consts = ctx.enter_context(tc.tile_pool(name="consts", bufs=1))
    iota = consts.tile([P, P], F32)
    nc.sync.dma_start(out=iota[:], in_=t["iota"][:, :])

    ipool = ctx.enter_context(tc.tile_pool(name="idx", bufs=3))
    gpool = ctx.enter_context(tc.tile_pool(name="g2", bufs=3))
    spool = ctx.enter_context(tc.tile_pool(name="s2", bufs=3))
    lpool = ctx.enter_context(tc.tile_pool(name="l2", bufs=2))
    mpool = ctx.enter_context(tc.tile_pool(name="m2", bufs=2))
    apsum = ctx.enter_context(tc.tile_pool(name="aps2", bufs=2,
                                           space="PSUM"))
    opool = ctx.enter_context(tc.tile_pool(name="o", bufs=3))

    tab_pair = _ap(t["table2"][:, :], [[256, cfg.N_PAD // 2], [1, 256]])

    for ti in range(TPC):
        c0 = ti * NCH_T
        isrc = ipool.tile([P, NCH_T * 8], I16, tag="isrc")
        nc.sync.dma_start(out=isrc[:], in_=t["srcw"][
            :, ti * NCH_T * 8:(ti + 1) * NCH_T * 8])
        dloc = ipool.tile([P, NCH_T], F32, tag="dloc")
        nc.sync.dma_start(out=dloc[:], in_=t["dstloc_f"][:, c0:c0 + NCH_T])
        spar = ipool.tile([P, NCH_T], F32, tag="spar")
        nc.sync.dma_start(out=spar[:], in_=t["src_par"][:, c0:c0 + NCH_T])
        # s2 of this tile's 128 dst nodes, replicated across partitions
        # (replication done on host; plain contiguous DMA)
        s2rep = ipool.tile([P, P], BF16, tag="s2rep")
        nc.sync.dma_start(out=s2rep[:],
                          in_=t["s2rows"][ti * P:(ti + 1) * P, :])

        G2 = gpool.tile([P, NCH_T, 256], BF16, tag="G2")
        nc.gpsimd.dma_gather(
            out_ap=G2[:], in_ap=tab_pair, idxs_ap=isrc[:],
            num_idxs=NCH_T * P, num_idxs_reg=NCH_T * P, elem_size=256,
            single_packet=False)

        # parity selects: x = odd + par*(even - odd)
        Rd = spool.tile([P, NCH_T, UC], F32, tag="Rd")
        nc.vector.tensor_tensor(out=Rd[:], in0=G2[:, :, 0:UC],
                                in1=G2[:, :, 128:128 + UC], op=OP.subtract)
        Rm = spool.tile([P, NCH_T, UC], F32, tag="Rm")
        nc.vector.tensor_tensor(
            out=Rm[:], in0=Rd[:],
            in1=_ap(spar[:], [spar[:].ap[0], [1, NCH_T], [0, UC]]),
            op=OP.mult)
        Ro = spool.tile([P, NCH_T, UC], F32, tag="Ro")
        nc.scalar.activation(Ro[:], G2[:, :, 128:128 + UC], AF.Copy)
        R = spool.tile([P, NCH_T, UC], F32, tag="R")
        nc.vector.tensor_tensor(out=R[:], in0=Rm[:], in1=Ro[:], op=OP.add)

        S = spool.tile([P, NCH_T, P], BF16, tag="S")
        nc.vector.tensor_tensor(
            out=S[:],
            in0=_ap(iota[:], [iota[:].ap[0], [0, NCH_T], [1, P]]),
            in1=_ap(dloc[:], [dloc[:].ap[0], [1, NCH_T], [0, P]]),
            op=OP.is_equal)

        # s2[dst] per edge: se2 = sum_j S[e,j] * s2rep[*,j] (one-hot pick)
        selm = spool.tile([P, NCH_T, P], BF16, tag="selm")
        nc.vector.tensor_tensor(
            out=selm[:], in0=S[:],
            in1=_ap(s2rep[:], [s2rep[:].ap[0], [0, NCH_T], [1, P]]),
            op=OP.mult)
        sesb = spool.tile([P, NCH_T, 1], F32, tag="sesb2")
        nc.vector.tensor_reduce(out=sesb[:], in_=selm[:],
                                axis=mybir.AxisListType.X, op=OP.add)

        u = lpool.tile([P, NCH_T, 1], F32, tag="u2")
        nc.vector.tensor_tensor(out=u[:], in0=sesb[:],
                                in1=R[:, :, cfg.OUT:MC], op=OP.add)
        a = lpool.tile([P, NCH_T, 1], F32, tag="a2")
        nc.vector.scalar_tensor_tensor(out=a[:], in0=u[:], scalar=cfg.NEG,
                                       in1=u[:], op0=OP.mult, op1=OP.max)
        ex = lpool.tile([P, NCH_T, 1], F32, tag="ex2")
        nc.scalar.activation(ex[:], a[:], AF.Exp)

        M = mpool.tile([P, NCH_T, MC], BF16, tag="M2")
        nc.scalar.activation(M[:, :, cfg.OUT:MC], ex[:], AF.Copy)
        nc.vector.tensor_tensor(
            out=M[:, :, 0:cfg.OUT],
            in0=R[:, :, 0:cfg.OUT],
            in1=_ap(ex[:], [ex[:].ap[0], [1, NCH_T], [0, cfg.OUT]]),
            op=OP.mult)

        agg = apsum.tile([P, MC], F32, tag="agg2")
        for k in range(NCH_T):
            nc.tensor.matmul(out=agg[:], lhsT=S[:, k, :], rhs=M[:, k, :],
                             start=(k == 0), stop=(k == NCH_T - 1))

        den = opool.tile([P, 1], F32, tag="den")
        nc.vector.tensor_scalar_add(den[:], agg[:, cfg.OUT:MC], 1e-20)
        rcp = opool.tile([P, 1], F32, tag="rcp")
        nc.vector.reciprocal(rcp[:], den[:])
        res = opool.tile([P, cfg.OUT + 1], F32, tag="res")
        nc.vector.tensor_tensor(
            out=res[:, 0:cfg.OUT], in0=agg[:, 0:cfg.OUT],
            in1=_ap(rcp[:], [rcp[:].ap[0], [0, cfg.OUT]]), op=OP.mult)
        # logits are O(0.3): exp() safe without max-subtraction; host
        # finishes log_softmax as h2 - log(sum_exp).
        pe = opool.tile([P, cfg.OUT], F32, tag="pe")
        nc.scalar.activation(pe[:], res[:, 0:cfg.OUT], AF.Exp,
                             accum_out=res[:, cfg.OUT:cfg.OUT + 1])
        nc.sync.dma_start(out=t["outp"][ti * P:(ti + 1) * P, :], in_=res[:])


def _decl_a1(nc, cfg):
    t = {}
    WCOLS = cfg.F + 2 * cfg.H

    def inp(name, shape, dt):
        t[name] = nc.dram_tensor(name, shape, dt, kind="ExternalInput").ap()

    inp("xTs", [P, cfg.NPC_PAD], F32)
    inp("wpack", [P, WCOLS], F32)
    t["t1slab"] = nc.dram_tensor("t1slab", [cfg.NPC_PAD, 256], BF16,
                                 kind="ExternalOutput").ap()
    t["sslab"] = nc.dram_tensor("sslab", [cfg.NPC_PAD, 8], BF16,
                                kind="ExternalOutput").ap()
    return t


def _decl_a2(nc, cfg):
    t = {}
    W2COLS = cfg.OUT + 2

    def inp(name, shape, dt):
        t[name] = nc.dram_tensor(name, shape, dt, kind="ExternalInput").ap()

    inp("table1", [cfg.N_PAD, 256], BF16)
    inp("sslab", [cfg.NPC_PAD, 8], BF16)
    inp("w2pack", [P, W2COLS], F32)
    inp("iota", [P, P], F32)
    inp("ident", [P, P], F32)
    inp("identb", [P, P], BF16)
    inp("srcw_lo", [P, cfg.TPC * cfg.NCHL_T * 8], I16)
    inp("srcw_hi", [P, cfg.TPC * cfg.NCHH_T * 8], I16)
    inp("dstloc_f", [P, cfg.NCH], F32)
    t["table2slab"] = nc.dram_tensor("table2slab", [cfg.NPC_PAD, W2COLS],
                                     F32, kind="ExternalOutput").ap()
    return t


def _decl_b(nc, cfg):
    t = {}

    def inp(name, shape, dt):
        t[name] = nc.dram_tensor(name, shape, dt, kind="ExternalInput").ap()

    inp("table2", [cfg.N_PAD, 128], BF16)
    inp("s2rows", [cfg.NPC_PAD, P], BF16)
    inp("srcw", [P, cfg.NCH * 8], I16)
    inp("dstloc_f", [P, cfg.NCH], F32)
    inp("src_par", [P, cfg.NCH], F32)
    inp("iota", [P, P], F32)
    t["outp"] = nc.dram_tensor("outp", [cfg.NPC_PAD, cfg.OUT + 1], F32,
                               kind="ExternalOutput").ap()
    return t


def _compile(build_fn, decl_fn, cfg):
    nc = bacc.Bacc("TRN2", target_bir_lowering=False, debug=False,
                   enable_asserts=False, num_devices=cfg.NCORES)
    t = decl_fn(nc, cfg)
    with tile.TileContext(nc) as tc:
        build_fn(tc, cfg, t)
    nc.compile()
    return nc


def _host_prep_weights(cfg, W1, att_src1, att_dst1, W2, att_src2, att_dst2):
    A_d1 = _blockdiag_att(np.asarray(att_dst1, np.float32), cfg.H, cfg.HID,
                          cfg.F)
    A_s1 = _blockdiag_att(np.asarray(att_src1, np.float32), cfg.H, cfg.HID,
                          cfg.F)
    W1T = np.asarray(W1, np.float32).T.copy()
    wpack = np.concatenate([W1T, W1T @ A_d1, W1T @ A_s1], axis=1)
    W2T = np.asarray(W2, np.float32).T.copy()
    a_d2 = np.asarray(att_dst2, np.float32).reshape(cfg.OUT, 1)
    a_s2 = np.asarray(att_src2, np.float32).reshape(cfg.OUT, 1)
    w2pack = np.concatenate([W2T, W2T @ a_d2, W2T @ a_s2], axis=1)
    return (np.ascontiguousarray(wpack, np.float32),
            np.ascontiguousarray(w2pack, np.float32))


_CACHE = {}


def _get_kernels(cfg):
    key = (cfg.N, cfg.E, cfg.NCORES, cfg.NCH_T, cfg.NCHL_T)
    if key not in _CACHE:
        nca1 = _compile(_build_a1, _decl_a1, cfg)
        nca2 = _compile(_build_a2, _decl_a2, cfg)
        ncb = _compile(_build_b, _decl_b, cfg)
        _CACHE[key] = (nca1, nca2, ncb)
    return _CACHE[key]


def run(cfg, inputs, runner=None):
    x = np.asarray(inputs["x"], np.float32)
    edge_index = np.asarray(inputs["edge_index"], np.int64)
    pc = _prep_graph(cfg, edge_index)
    wpack, w2pack = _host_prep_weights(
        cfg, inputs["W1"], inputs["att_src1"], inputs["att_dst1"],
        inputs["W2"], inputs["att_src2"], inputs["att_dst2"])

    xT = np.zeros((P, cfg.N_PAD), np.float32)
    xT[:, :cfg.N] = x.T
    iota = np.tile(np.arange(P, dtype=np.float32), (P, 1))
    ident = np.eye(P, dtype=np.float32)
    identb = np.eye(P, dtype=BF)

    nca1, nca2, ncb = _get_kernels(cfg)

    if runner is None:
        def runner(nc, in_maps):
            r = bass_utils.run_bass_kernel_spmd(
                nc, in_maps, core_ids=list(range(cfg.NCORES)))
            return r.results

    # --- A1: sharded node phase ---
    in_maps_a1 = []
    for c in range(cfg.NCORES):
        xTs = np.zeros((P, cfg.NPC_PAD), np.float32)
        n0 = c * cfg.NPC
        n1 = min(n0 + cfg.NPC_PAD, cfg.N_PAD)
        xTs[:, 0:n1 - n0] = xT[:, n0:n1]
        in_maps_a1.append(dict(xTs=xTs, wpack=wpack))
    res_a1 = runner(nca1, in_maps_a1)

    table1 = np.zeros((cfg.N_PAD, 256), BF)
    sslabs = []
    for c in range(cfg.NCORES):
        t1s = np.asarray(res_a1[c]["t1slab"])
        n0 = c * cfg.NPC
        n1 = min(n0 + cfg.NPC_PAD, cfg.N_PAD)
        take = min(cfg.NPC, n1 - n0)
        table1[n0:n0 + take] = t1s[:take]
        sslabs.append(np.asarray(res_a1[c]["sslab"]))

    # --- A2: layer-0 edge phase ---
    in_maps_a2 = []
    for c in range(cfg.NCORES):
        in_maps_a2.append(dict(
            table1=table1, sslab=sslabs[c], w2pack=w2pack, iota=iota,
            ident=ident, identb=identb,
            srcw_lo=pc["srcw_lo"][c], srcw_hi=pc["srcw_hi"][c],
            dstloc_f=pc["dstloc_f"][c]))
    res_a2 = runner(nca2, in_maps_a2)

    table2 = np.zeros((cfg.N_PAD, 128), BF)
    s2slabs = []
    for c in range(cfg.NCORES):
        slab = np.asarray(res_a2[c]["table2slab"], np.float32)
        table2[c * cfg.NPC:(c + 1) * cfg.NPC, 0:cfg.OUT + 2] = \
            slab[:cfg.NPC].astype(BF)
        s2row = slab[:, cfg.OUT + 1].astype(BF).reshape(cfg.TPC, 1, P)
        s2slabs.append(np.ascontiguousarray(
            np.broadcast_to(s2row, (cfg.TPC, P, P))
            .reshape(cfg.NPC_PAD, P)))

    # --- B: layer-1 edge phase ---
    in_maps_b = []
    for c in range(cfg.NCORES):
        in_maps_b.append(dict(
            table2=table2, s2rows=s2slabs[c], srcw=pc["srcw"][c],
            dstloc_f=pc["dstloc_f"][c], src_par=pc["src_par"][c],
            iota=iota))
    res_b = runner(ncb, in_maps_b)

    out = np.zeros((cfg.N, cfg.OUT), np.float32)
    for c in range(cfg.NCORES):
        ob = np.asarray(res_b[c]["outp"], np.float32)[:cfg.NPC]
        out[c * cfg.NPC:(c + 1) * cfg.NPC] = \
            ob[:, 0:cfg.OUT] - np.log(ob[:, cfg.OUT:cfg.OUT + 1])
    return out


def kernel(**inputs):
    cfg = Cfg(N=50000, E=1600000, ncores=8)
    return run(cfg, inputs)
